# revision 1
# baseline (speedup 1.0000x reference)
"""Dilated self-attention TRN2 Bass kernel.

Problem (hardcoded): B=2, N=8192, C=256, WS=[2048,4096,8192], RS=[1,2,4],
HEAD_IDX=0 -> G=7 groups of s=2048 rows each.

Sharding: 8 cores, core d = (b=d//4, q=d%4) owns output positions
[2048q, 2048(q+1)) of batch b.  Each core computes, fully locally:
  job A: the r=1 segment group g_q of its quarter (2048 queries, causal)
  job B: the 1024-query half of the r=2 group whose outputs land in its quarter
  job C: the 512-query quarter of the r=4 group landing in its quarter
Attention is computed unnormalized: U = exp(scores/16) @ [V | 1], so the last
column carries the softmax denominator.  The cross-group scatter-add combine is
local: U_B rows scatter into the quarter at stride 2, U_C at stride 4, done
with constant 0/1 placement-matrix matmuls (Z = P.T @ U lands rows on the
right partitions) plus lane-aligned DVE adds into a resident, pre-zeroed SBUF
accumulator; then out = U[:, :256] / U[:, 256].  Jobs run C, B, A so the final
dependency chain is short.  Host does only gather/transpose/zero-pad (sharding) and
concatenation (unsharding) - no arithmetic.

The SPMD program is identical on all cores; per-core differences are carried by
input data only (pre-gathered transposed slabs + bias vectors that disable
padded prefix k-tiles via exp's bias = -1e9).
"""

import numpy as np

B, N, C = 2, 8192, 256
S = 2048          # rows per group
NCORES = 8
SCALE = 0.0625    # 1/sqrt(256)
NEG = -1.0e9

_PROG = None      # cached compiled Bass program


def _build_program(mm_fp32=False):
    import concourse.mybir as mybir
    import concourse.tile as tile
    from concourse import bacc

    F32 = mybir.dt.float32
    MMDT = mybir.dt.float32 if mm_fp32 else mybir.dt.float32r
    Exp = mybir.ActivationFunctionType.Exp

    nc = bacc.Bacc("TRN2", target_bir_lowering=False, debug=False,
                   num_devices=NCORES)

    xA = nc.dram_tensor("xA", [C, S], MMDT, kind="ExternalInput")
    xB = nc.dram_tensor("xB", [C, S], MMDT, kind="ExternalInput")
    xC = nc.dram_tensor("xC", [C, S], MMDT, kind="ExternalInput")
    w_d = nc.dram_tensor("w", [C, 3 * C], MMDT, kind="ExternalInput")
    bias_d = nc.dram_tensor("bias", [128, 20], F32, kind="ExternalInput")
    out_d = nc.dram_tensor("out", [S, C], F32, kind="ExternalOutput")

    # job specs: (x dram, n q rows, q row offset in slab, prefix k-tiles,
    #             bias dram or None, scatter stride)
    jobs = [
        dict(x=xC, nq=512, q0=1536, P=12, bias=8, stride=4),
        dict(x=xB, nq=1024, q0=1024, P=8, bias=0, stride=2),
        dict(x=xA, nq=2048, q0=0, P=0, bias=None, stride=1),
    ]

    with tile.TileContext(nc) as tc:
        with (
            tc.tile_pool(name="const", bufs=1) as cpool,
            tc.tile_pool(name="xsb", bufs=2) as xpool,
            tc.tile_pool(name="ktp", bufs=3) as ktpool,
            tc.tile_pool(name="qtp", bufs=2) as qtpool,
            tc.tile_pool(name="vext", bufs=20) as vpool,
            tc.tile_pool(name="probs", bufs=24) as ppool,
            tc.tile_pool(name="stage", bufs=4) as spool,
            tc.tile_pool(name="acc", bufs=1) as apool,
            tc.tile_pool(name="fin", bufs=4) as fpool,
            tc.tile_pool(name="ps_s", bufs=3, space="PSUM") as ps_scores,
            tc.tile_pool(name="ps_u", bufs=3, space="PSUM") as ps_u,
            tc.tile_pool(name="ps_p", bufs=2, space="PSUM") as ps_proj,
        ):
            # ---- constants (weights first: every projection needs them) ----
            w_sb = {}   # (name, ci) -> AP [128, 256]
            wt = []
            for ci in range(2):
                t = cpool.tile([128, 3 * C], MMDT, tag=f"wt{ci}", name=f"wt{ci}")
                eng = nc.sync if ci == 0 else nc.scalar
                eng.dma_start(t[:], w_d[128 * ci:128 * (ci + 1), :])
                wt.append(t)

            # ---- first job's first chunks (start PE asap) ----
            slab = {}   # (jn, ci) -> list of (tile, col offset)
            for ci in range(2):
                t = xpool.tile([128, 512], MMDT, tag="xsb", name=f"xa0_{ci}")
                eng = nc.sync if ci == 0 else nc.scalar
                eng.dma_start(t[:], jobs[0]["x"][128 * ci:128 * (ci + 1), 0:512])
                slab[0, ci] = [(t, 0)]
            for i, nm in enumerate(("q", "k", "v")):
                for ci in range(2):
                    w_sb[nm, ci] = wt[ci][:, 256 * i:256 * (i + 1)]
            bias_t = cpool.tile([128, 20], F32, tag="bias")
            nc.sync.dma_start(bias_t[:], bias_d[:])

            # ---- remaining slab loads ----
            for ci in range(2):
                t = xpool.tile([128, 1536], MMDT, tag="xsb1", name=f"xa1_{ci}")
                eng = nc.sync if ci == 0 else nc.scalar
                eng.dma_start(t[:], jobs[0]["x"][128 * ci:128 * (ci + 1), 512:2048])
                slab[0, ci].append((t, 512))
            for jn2 in (1, 2):
                xd = jobs[jn2]["x"]   # prefetch in job order
                for ci in range(2):
                    t = xpool.tile([128, 2048], MMDT, bufs=4, tag="xsb2",
                                   name=f"x{jn2}_{ci}")
                    eng = nc.sync if ci == 0 else nc.scalar
                    eng.dma_start(t[:], xd[128 * ci:128 * (ci + 1), :])
                    slab[jn2, ci] = [(t, 0)]

            def xslice(jn2, ci, c0, c1):
                for t, off in reversed(slab[jn2, ci]):
                    if c0 >= off:
                        assert c1 - off <= t.shape[-1], (jn2, ci, c0, c1)
                        return t[:, c0 - off:c1 - off]
                raise AssertionError

            ones_t = cpool.tile([128, 128], F32, tag="ones")
            nc.gpsimd.memset(ones_t[:], 1.0)
            ones01 = cpool.tile([128, 2], F32, tag="ones01")
            nc.vector.memset(ones01[:, 0:1], 1.0)
            nc.vector.memset(ones01[:, 1:2], 0.0)
            mtri_f = cpool.tile([128, 128], F32, tag="fscratch", name="mtri_f")
            nc.gpsimd.affine_select(
                out=mtri_f[:], in_=ones_t[:],
                compare_op=mybir.AluOpType.is_ge,
                fill=0.0, base=0,
                pattern=[[1, 128]], channel_multiplier=-1,
            )
            mtri = cpool.tile([128, 128], MMDT, tag="mtri")
            nc.vector.tensor_copy(mtri[:], mtri_f[:])

            # placement matrices: P[m, p] = 1 iff p == stride*m - 128*u
            # (Z = P.T @ U puts U row m onto partition stride*m - 128*u)
            pmats = {}
            for stride, u in [(2, 0), (2, 1), (4, 0), (4, 1), (4, 2), (4, 3)]:
                pf = cpool.tile([128, 128], F32, tag="fscratch",
                                name=f"pmf{stride}_{u}")
                nc.gpsimd.affine_select(
                    out=pf[:], in_=ones_t[:],
                    compare_op=mybir.AluOpType.is_equal,
                    fill=0.0, base=128 * u,
                    pattern=[[1, 128]], channel_multiplier=-stride,
                )
                pm = cpool.tile([128, 128], MMDT, tag=f"pm{stride}_{u}",
                                name=f"pm{stride}_{u}")
                nc.vector.tensor_copy(pm[:], pf[:])
                pmats[stride, u] = pm

            # persistent accumulator: 16 tiles of [128, 257] covering the
            # 2048 output positions of this core's quarter
            acc = [apool.tile([128, 257], F32, tag=f"acc{t}", name=f"acc{t}")
                   for t in range(16)]
            for t in range(16):
                nc.vector.memset(acc[t][:], 0.0)

            # ---- jobs ----
            for jn, job in enumerate(jobs):
                nq, q0, P = job["nq"], job["q0"], job["P"]
                stride = job["stride"]
                nkt_all = 16          # k/v tiles per job (always full slab)


                # projections, emitted in chunk waves so PE can start
                # as soon as the first 512-column slab chunk lands
                kt_sb = [ktpool.tile([128, S], MMDT, tag="kt", name=f"kt{jn}_{_i}")
                         for _i in range(2)]
                qt_sb = [qtpool.tile([128, nq], MMDT, tag="qt", name=f"qt{jn}_{_i}")
                         for _i in range(2)]
                vext = [None] * nkt_all
                for kc in range(4):
                    for co in range(2):
                        ps = ps_proj.tile([128, 512], F32, tag="proj")
                        for ci in range(2):
                            nc.tensor.matmul(
                                ps[:], w_sb["k", ci][:, 128 * co:128 * (co + 1)],
                                xslice(jn, ci, 512 * kc, 512 * (kc + 1)),
                                start=(ci == 0), stop=(ci == 1))
                        nc.scalar.copy(
                            kt_sb[co][:, 512 * kc:512 * (kc + 1)], ps[:])
                    if 512 * kc >= q0:
                        qc = (512 * kc - q0) // 512
                        for co in range(2):
                            ps = ps_proj.tile([128, 512], F32, tag="proj")
                            for ci in range(2):
                                nc.tensor.matmul(
                                    ps[:], w_sb["q", ci][:, 128 * co:128 * (co + 1)],
                                    xslice(jn, ci, q0 + 512 * qc,
                                           q0 + 512 * (qc + 1)),
                                    start=(ci == 0), stop=(ci == 1))
                            nc.vector.tensor_copy(
                                qt_sb[co][:, 512 * qc:512 * (qc + 1)], ps[:])
                    for kt in range(4 * kc, 4 * kc + 4):
                        ps = ps_proj.tile([128, 256], F32, tag="proj", name="psv")
                        for ci in range(2):
                            nc.tensor.matmul(
                                ps[:], xslice(jn, ci, 128 * kt, 128 * (kt + 1)),
                                w_sb["v", ci][:],
                                start=(ci == 0), stop=(ci == 1))
                        v = vpool.tile([128, 258], MMDT, tag="vext")
                        nc.vector.tensor_copy(v[:, 0:256], ps[:])
                        nc.vector.tensor_copy(v[:, 256:258], ones01[:])
                        vext[kt] = v

                # ---- attention over 512-wide q blocks ----
                for i in range(nq // 512):
                    nkt = P + 4 * i + 4
                    probs = []      # (tile, qoff) per k-tile
                    for kt in range(nkt):
                        jd = kt - (P + 4 * i)
                        # diag tiles only need q columns >= 128*jd; clamp to
                        # >=256 wide to keep the f32r full-rate path
                        qoff = 0 if jd < 0 else min(128 * jd, 256)
                        w = 512 - qoff
                        ps = ps_scores.tile([128, 512], F32, tag="scores")
                        for ci in range(2):
                            nc.tensor.matmul(
                                ps[:, 0:w], kt_sb[ci][:, 128 * kt:128 * (kt + 1)],
                                qt_sb[ci][:, 512 * i + qoff:512 * (i + 1)],
                                start=(ci == 0), stop=(ci == 1))
                        pb = ppool.tile([128, 512], MMDT, tag="probs")
                        if kt < P:
                            b0 = job["bias"]
                            bias_ap = bias_t[:, b0 + kt:b0 + kt + 1]
                        else:
                            bias_ap = 0.0
                        nc.scalar.activation(pb[:, 0:w], ps[:, 0:w], Exp,
                                             bias=bias_ap, scale=SCALE)
                        if jd >= 0:
                            c0 = 128 * jd - qoff
                            nc.vector.tensor_mul(
                                pb[:, c0:c0 + 128],
                                pb[:, c0:c0 + 128], mtri[:])
                        probs.append((pb, qoff))

                    for j in range(4):
                        nk = P + 4 * i + j + 1
                        ups = ps_u.tile([128, 258], F32, tag="u")
                        for kk in range(nk):
                            pb, qoff = probs[kk]
                            c0 = 128 * j - qoff
                            nc.tensor.matmul(
                                ups[:], pb[:, c0:c0 + 128],
                                vext[kk][:],
                                start=(kk == 0), stop=(kk == nk - 1))
                        t_local = 4 * i + j  # q tile index within job
                        if stride == 1:
                            nc.vector.tensor_add(acc[t_local][:], acc[t_local][:],
                                                 ups[:, 0:257])
                        else:
                            st = spool.tile([128, 258], MMDT, tag="stage")
                            with tc.high_priority(offset=20):
                                nc.vector.tensor_copy(st[:], ups[:])
                            with tc.high_priority(offset=-40):
                                for u in range(stride):
                                    zps = ps_u.tile([128, 258], F32, tag="u",
                                                    name="zps")
                                    nc.tensor.matmul(zps[:], pmats[stride, u][:],
                                                     st[:], start=True, stop=True)
                                    at = acc[stride * t_local + u]
                                    nc.vector.tensor_add(at[:], at[:],
                                                         zps[:, 0:257])

                
            # ---- finalize: divide by denominator, store ----
            fin = [fpool.tile([128, 2, 256], F32, bufs=1, tag=f"fing{g}", name=f"fing{g}")
                   for g in range(8)]
            out_r = out_d.rearrange("(g t p) c -> g p t c", p=128, t=2)
            for t in range(16):
                g, ti = divmod(t, 2)
                rec = fpool.tile([128, 1], F32, tag="rec")
                nc.vector.reciprocal(rec[:], acc[t][:, 256:257])
                nc.vector.tensor_scalar_mul(fin[g][:, ti, :],
                                            acc[t][:, 0:256], rec[:])
                if ti == 1:
                    eng = nc.sync if g % 2 == 0 else nc.scalar
                    eng.dma_start(out_r[g], fin[g][:])

    nc.compile()
    return nc


def _get_program():
    global _PROG
    if _PROG is None:
        _PROG = _build_program()
    return _PROG


def make_in_maps(x, Wq, Wk, Wv):
    """Host-side sharding: pure gather / transpose / zero-pad, no arithmetic."""
    x = np.asarray(x, dtype=np.float32)
    Wq = np.ascontiguousarray(np.asarray(Wq, dtype=np.float32))
    Wk = np.ascontiguousarray(np.asarray(Wk, dtype=np.float32))
    Wv = np.ascontiguousarray(np.asarray(Wv, dtype=np.float32))
    in_maps = []
    for d in range(NCORES):
        b, q = divmod(d, 4)
        xA = np.ascontiguousarray(x[b, 2048 * q:2048 * (q + 1), :].T)

        seg = 0 if q < 2 else 4096
        grp2 = x[b, seg:seg + 4096:2, :]          # [2048, 256]
        r0 = 1024 * (q % 2)
        if r0 == 1024:
            rowsB = grp2                           # prefix real + diag
        else:
            rowsB = np.concatenate(
                [np.zeros((1024, C), np.float32), grp2[0:1024]], axis=0)
        xB = np.ascontiguousarray(rowsB.T)

        grp4 = x[b, 0:8192:4, :]                  # [2048, 256]
        r0c = 512 * q
        rowsC = np.concatenate(
            [grp4[0:r0c], np.zeros((1536 - r0c, C), np.float32),
             grp4[r0c:r0c + 512]], axis=0)
        xC = np.ascontiguousarray(rowsC.T)

        bias = np.zeros((128, 20), np.float32)
        bias[:, 0:8] = 0.0 if r0 == 1024 else NEG
        bias[:, 8 + 4 * q:20] = NEG

        in_maps.append({
            "xA": xA, "xB": xB, "xC": xC,
            "w": np.ascontiguousarray(np.concatenate([Wq, Wk, Wv], axis=1)),
            "bias": bias,
        })
    return in_maps


def kernel(x, Wq, Wk, Wv):
    from concourse.bass_utils import run_bass_kernel_spmd

    nc = _get_program()
    in_maps = make_in_maps(x, Wq, Wk, Wv)
    res = run_bass_kernel_spmd(nc, in_maps, core_ids=list(range(NCORES)))
    out = np.empty((B, N, C), np.float32)
    for d in range(NCORES):
        b, q = divmod(d, 4)
        out[b, 2048 * q:2048 * (q + 1), :] = res.results[d]["out"]
    return out



# revision 2
# speedup vs baseline: 1.1145x; 1.1145x over previous
"""Dilated self-attention TRN2 Bass kernel, v2 (bf16, exact causal shapes).

Problem (hardcoded): B=2, N=8192, C=256, WS=[2048,4096,8192], RS=[1,2,4],
HEAD_IDX=0 -> G=7 groups of s=2048 rows each.

Sharding: core d = (b=d//4, q=d%4) owns output positions [2048q, 2048(q+1))
of batch b.  Jobs per core (all bf16 matmuls, 1 cycle/row at any width):
  C: 4 q-tiles (slots 12..15) of the r=4 group, prefix 12 kt (data-masked)
  B: 8 q-tiles (slots 8..15) of the r=2 half, prefix 8 kt (data-masked)
  A: 16 q-tiles full causal over own quarter
Layout tricks vs v1:
  - One bf16 x slab [128,2,4608] = [A 2048 | Bpre 1024 | Cpre 1536]; B/C
    diag K/Q are strided views into the A region (no re-projection).
  - V tiles for B/C diag are partition-gathered from A's V tiles via
    SBUF->SBUF DMA (stride 2/4) instead of re-projecting.
  - Exact per-q-tile causal prefixes (no 512-block rounding).
  - Scatter-add runs inside the output PSUM accumulation group: placement
    matmuls for B/C land before A's U matmuls; finalize reads PSUM once.
  - Emission order starts the C job while A/B projections still run.
"""

import numpy as np

B, N, C = 2, 8192, 256
S = 2048
NCORES = 8
SCALE = 0.0625
NEG = -1.0e9

XA, XB, XC = 2048, 1024, 1536          # slab section widths
XW = XA + XB + XC                      # 4608

_PROG = None


def _build_program():
    import concourse.mybir as mybir
    import concourse.tile as tile
    from concourse import bacc

    F32 = mybir.dt.float32
    BF16 = mybir.dt.bfloat16
    Exp = mybir.ActivationFunctionType.Exp

    nc = bacc.Bacc("TRN2", target_bir_lowering=False, debug=False,
                   num_devices=NCORES)

    x_d = nc.dram_tensor("x", [128, 2 * XW], BF16, kind="ExternalInput")
    w_d = nc.dram_tensor("w", [128, 2 * 3 * C], BF16, kind="ExternalInput")
    bias_d = nc.dram_tensor("bias", [128, 4], F32, kind="ExternalInput")
    out_d = nc.dram_tensor("out", [S, C], F32, kind="ExternalOutput")

    with tile.TileContext(nc) as tc:
        with (
            tc.tile_pool(name="const", bufs=1) as cpool,
            tc.tile_pool(name="xsb", bufs=1) as xpool,
            tc.tile_pool(name="kqt", bufs=1) as kqpool,
            tc.tile_pool(name="vext", bufs=24) as vpool,
            tc.tile_pool(name="probs", bufs=4) as ppool,
            tc.tile_pool(name="probs2", bufs=4) as ppool2,
            tc.tile_pool(name="stage", bufs=12) as spool,
            tc.tile_pool(name="fin", bufs=4) as fpool,
            tc.tile_pool(name="ps_s", bufs=3, space="PSUM") as ps_scores,
            tc.tile_pool(name="ps_u", bufs=2, space="PSUM") as ps_small,
        ):
            # ---- weights + early x chunks first (PE warmup) ----
            w_sb = cpool.tile([128, 2, 3 * C], BF16, tag="w")
            nc.sync.dma_start(w_sb[:], w_d.rearrange("p (c k) -> p c k", c=2))

            xb = xpool.tile([128, 2, XW], BF16, tag="xb")
            x_r = x_d.rearrange("p (c k) -> p c k", c=2)
            CH = 512
            CHUNK_ORDER = [0, 1, 6, 7, 8, 2, 3, 4, 5]
            for i, ch in enumerate(CHUNK_ORDER):
                eng = nc.sync if i % 2 == 0 else nc.scalar
                eng.dma_start(xb[:, :, CH * ch:CH * (ch + 1)],
                              x_r[:, :, CH * ch:CH * (ch + 1)])
            bias_t = cpool.tile([128, 4], F32, tag="bias")
            nc.sync.dma_start(bias_t[:], bias_d[:])

            def wap(name, ci, co):
                i = {"q": 0, "k": 1, "v": 2}[name]
                return w_sb[:, ci, 256 * i + 128 * co:256 * i + 128 * (co + 1)]

            # ---- constants ----
            ones_t = cpool.tile([128, 128], F32, tag="ones")
            nc.gpsimd.memset(ones_t[:], 1.0)
            # identity (stationary) and -1e9 strict-upper (moving): adding
            # ident.T @ trineg to a diag score tile applies the causal mask
            # inside the PSUM accumulation group, PE-side.
            id_f = cpool.tile([128, 128], F32, tag="id_f")
            nc.gpsimd.affine_select(
                out=id_f[:], in_=ones_t[:],
                compare_op=mybir.AluOpType.is_equal,
                fill=0.0, base=0, pattern=[[1, 128]], channel_multiplier=-1)
            ident = cpool.tile([128, 128], BF16, tag="ident")
            nc.vector.tensor_copy(ident[:], id_f[:])
            neg_t = cpool.tile([128, 128], F32, tag="negs")
            nc.gpsimd.memset(neg_t[:], NEG)
            tri_f = cpool.tile([128, 128], F32, tag="tri_f")
            # keep NEG where p > f (mask keys k > query q), 0 elsewhere
            nc.gpsimd.affine_select(
                out=tri_f[:], in_=neg_t[:],
                compare_op=mybir.AluOpType.is_ge,
                fill=0.0, base=-1, pattern=[[-1, 128]], channel_multiplier=1)
            trineg = cpool.tile([128, 128], BF16, tag="trineg")
            nc.vector.tensor_copy(trineg[:], tri_f[:])

            pmats = {}
            for stride, u in [(2, 0), (2, 1), (4, 0), (4, 1), (4, 2), (4, 3)]:
                pf = cpool.tile([128, 128], F32, tag="fscratch",
                                name=f"pmf{stride}_{u}")
                nc.gpsimd.affine_select(
                    out=pf[:], in_=ones_t[:],
                    compare_op=mybir.AluOpType.is_equal,
                    fill=0.0, base=128 * u,
                    pattern=[[1, 128]], channel_multiplier=-stride)
                pm = cpool.tile([128, 128], BF16, tag=f"pm{stride}_{u}")
                nc.vector.tensor_copy(pm[:], pf[:])
                pmats[stride, u] = pm

            # ---- projection helpers (emitted lazily, in job-driven order) --
            ktb = kqpool.tile([128, 2, XW], BF16, tag="ktb")
            qtb = kqpool.tile([128, 2, XA], BF16, tag="qtb")
            _cp_flip = [0]

            def copy_to(dst, src):
                _cp_flip[0] ^= 1
                if _cp_flip[0]:
                    nc.scalar.copy(dst, src)
                else:
                    nc.vector.tensor_copy(dst, src)

            def kproj(ch):
                c0 = CH * ch
                for co in range(2):
                    ps = ps_small.tile([128, 512], F32, tag="proj", bufs=3)
                    for ci in range(2):
                        nc.tensor.matmul(
                            ps[:], wap("k", ci, co), xb[:, ci, c0:c0 + CH],
                            start=(ci == 0), stop=(ci == 1))
                    copy_to(ktb[:, co, c0:c0 + CH], ps[:])

            def qproj(ch):
                c0 = CH * ch
                for co in range(2):
                    ps = ps_small.tile([128, 512], F32, tag="proj", bufs=3)
                    for ci in range(2):
                        nc.tensor.matmul(
                            ps[:], wap("q", ci, co), xb[:, ci, c0:c0 + CH],
                            start=(ci == 0), stop=(ci == 1))
                    copy_to(qtb[:, co, c0:c0 + CH], ps[:])

            def vproj_pair(src_c0, name):
                """project x cols [src_c0, src_c0+256) -> one [128,2,258]"""
                ps = ps_small.tile([128, 512], F32, tag="proj", bufs=3,
                                   name=f"psv{name}")
                for half in range(2):
                    for ci in range(2):
                        nc.tensor.matmul(
                            ps[:, 256 * half:256 * (half + 1)],
                            xb[:, ci,
                               src_c0 + 128 * half:src_c0 + 128 * (half + 1)],
                            w_sb[:, ci, 512:768],
                            start=(ci == 0), stop=(ci == 1))
                v = vpool.tile([128, 2, 258], BF16, tag="vp", name=f"vp{name}")
                copy_to(v[:, :, 0:256],
                        ps.rearrange("p (h c) -> p h c", h=2))
                nc.vector.memset(v[:, :, 256:257], 1.0)
                nc.vector.memset(v[:, :, 257:258], 0.0)
                return v

            vA = [None] * 8

            def vAp(j):
                if vA[j] is None:
                    vA[j] = vproj_pair(256 * j, f"A{j}")
                return vA[j]

            def vBd_gather(j):
                """B diag pair j rows = A rows [512j, 512j+512) step 2."""
                v = vpool.tile([128, 2, 258], BF16, tag="vp", name=f"vBd{j}")
                for half in range(2):
                    for s in range(2):
                        src = vAp(2 * j + half)
                        nc.sync.dma_start(
                            v[64 * s:64 * (s + 1), half, :],
                            src[0:128:2, s, :])
                return v

            def vCd_gather(j):
                """C diag pair j rows = A rows [1024j, 1024j+1024) step 4."""
                v = vpool.tile([128, 2, 258], BF16, tag="vp", name=f"vCd{j}")
                for half in range(2):
                    base = 512 * (2 * j + half)
                    for s in range(4):
                        src = vAp((base + 128 * s) // 256)
                        sh = ((base + 128 * s) // 128) % 2
                        nc.sync.dma_start(
                            v[32 * s:32 * (s + 1), half, :],
                            src[0:128:4, sh, :])
                return v

            jobs = {
                "C": dict(nq=4, P=12, stride=4, kpre=XA + XB, dstep=4),
                "B": dict(nq=8, P=8, stride=2, kpre=XA, dstep=2),
                "A": dict(nq=16, P=0, stride=1, kpre=None, dstep=1),
            }
            vslabs = {}

            def kstat(job, ci, kt):
                j = jobs[job]
                if kt < j["P"]:
                    c0 = j["kpre"] + 128 * kt
                    return ktb[:, ci, c0:c0 + 128]
                d = kt - j["P"]
                st = j["dstep"]
                return ktb[:, ci, 128 * st * d:128 * st * (d + 1):st]

            def qmov(job, ci, tau):
                st = jobs[job]["dstep"]
                return qtb[:, ci, 128 * st * tau:128 * st * (tau + 1):st]

            def vmov(job, kt):
                j = vslabs[job][kt // 2]
                return j[:, kt % 2, :]

            st_tiles = {}

            def do_qtile(job, tau, ups):
                """scores+exp+U for q-tile tau of job; U accumulated into
                psum tile `ups` (A: group already started by placements)."""
                j = jobs[job]
                P = j["P"]
                nkt = P + tau + 1
                pbs = []            # (pb_tile, kt0, nkt_in_group)
                for gi, g0 in enumerate(range(0, nkt, 4)):
                    gn = min(4, nkt - g0)
                    ps = ps_scores.tile([128, 512], F32, tag="scores")
                    for i in range(gn):
                        kt = g0 + i
                        diag = kt == nkt - 1
                        reg = ps[:, 128 * i:128 * (i + 1)]
                        for ci in range(2):
                            nc.tensor.matmul(
                                reg, kstat(job, ci, kt), qmov(job, ci, tau),
                                start=(ci == 0), stop=(ci == 1 and not diag))
                        if diag:
                            nc.tensor.matmul(reg, ident[:], trineg[:],
                                             start=False, stop=True)
                    # one exp per 4-kt group (P is a multiple of 4, so a
                    # group is uniformly prefix or diag -> single bias)
                    if g0 < P:
                        b0 = 0 if job == "B" else 1 + g0 // 4
                        bias_ap = bias_t[:, b0:b0 + 1]
                    else:
                        bias_ap = 0.0
                    pb = (ppool if gi % 2 == 0 else ppool2).tile(
                        [128, 512], BF16, tag="pb")
                    pbs.append((pb, g0, gn))
                    nc.scalar.activation(pb[:, 0:128 * gn], ps[:, 0:128 * gn],
                                         Exp, bias=bias_ap, scale=SCALE)
                for pb, g0, gn in pbs:
                    for i in range(gn):
                        kt = g0 + i
                        nc.tensor.matmul(
                            ups[:], pb[:, 128 * i:128 * (i + 1)],
                            vmov(job, kt),
                            start=(kt == 0 and job != "A"),
                            stop=(kt == nkt - 1))

            def run_shared(job, tau):
                ups = ps_small.tile([128, 258], F32, tag="u",
                                    name=f"u{job}{tau}")
                do_qtile(job, tau, ups)
                st = spool.tile([128, 258], BF16, tag="st",
                                name=f"st{job}{tau}")
                nc.vector.tensor_copy(st[:], ups[:])
                st_tiles[job, tau] = st

            # ================= emission schedule =================
            # projections needed for C first, C starts ASAP, then B, then A.
            kproj(0); qproj(0)
            vAp(0); vAp(1)
            kproj(1); qproj(1)
            vAp(2); vAp(3)
            for ch in (6, 7, 8):
                kproj(ch)
            vCpre = [vproj_pair(XA + XB + 256 * j, f"Cp{j}") for j in range(6)]
            vslabC = vCpre + [vCd_gather(0)]
            vslabs["C"] = vslabC           # vCd[1] appended below

            run_shared("C", 0)
            kproj(2); qproj(2)
            vAp(4); vAp(5)
            run_shared("C", 1)
            kproj(3); qproj(3)
            vAp(6); vAp(7)
            vslabC.append(vCd_gather(1))
            run_shared("C", 2)
            kproj(4); kproj(5)
            vBpre = [vproj_pair(XA + 256 * j, f"Bp{j}") for j in range(4)]
            run_shared("C", 3)
            vBd = [vBd_gather(j) for j in range(4)]
            vslabs["B"] = vBpre + vBd
            vslabs["A"] = vA

            # ---- B job interleaved with A (A tile t needs st_B[t//2]) ----
            fin = [fpool.tile([128, 2, 256], F32, bufs=1, tag=f"fing{g}",
                              name=f"fing{g}") for g in range(8)]
            out_r = out_d.rearrange("(g t p) c -> g p t c", p=128, t=2)

            def run_A(t):
                ups = ps_small.tile([128, 258], F32, tag="u", name=f"uA{t}")
                nc.tensor.matmul(ups[:], pmats[4, t % 4][:],
                                 st_tiles["C", t // 4][:],
                                 start=True, stop=False)
                nc.tensor.matmul(ups[:], pmats[2, t % 2][:],
                                 st_tiles["B", t // 2][:],
                                 start=False, stop=False)
                do_qtile("A", t, ups)
                g, ti = divmod(t, 2)
                rec = fpool.tile([128, 1], F32, tag="rec")
                nc.vector.reciprocal(rec[:], ups[:, 256:257])
                nc.vector.tensor_scalar_mul(fin[g][:, ti, :],
                                            ups[:, 0:256], rec[:])
                if ti == 1:
                    eng = nc.sync if g % 2 == 0 else nc.scalar
                    eng.dma_start(out_r[g], fin[g][:])

            for tau in range(8):
                run_shared("B", tau)
                run_A(2 * tau)
                run_A(2 * tau + 1)

    nc.compile()
    return nc


def _get_program():
    global _PROG
    if _PROG is None:
        _PROG = _build_program()
    return _PROG


def make_in_maps(x, Wq, Wk, Wv):
    """Host-side sharding: gather / transpose / zero-pad / dtype cast only."""
    import ml_dtypes
    bf16 = np.dtype(ml_dtypes.bfloat16)
    x = np.asarray(x, dtype=np.float32)
    w_all = np.concatenate([np.asarray(Wq, np.float32),
                            np.asarray(Wk, np.float32),
                            np.asarray(Wv, np.float32)], axis=1)  # [C, 3C]
    w_t = np.ascontiguousarray(
        w_all.reshape(2, 128, 3 * C).transpose(1, 0, 2).reshape(128, 6 * C)
    ).astype(bf16)

    in_maps = []
    for d in range(NCORES):
        b, q = divmod(d, 4)
        quarter = x[b, 2048 * q:2048 * (q + 1), :]          # [2048, C]

        seg = 0 if q < 2 else 4096
        grp2 = x[b, seg:seg + 4096:2, :]                    # [2048, C]
        r0 = 1024 * (q % 2)
        bpre = grp2[0:1024] if r0 == 1024 else np.zeros((XB, C), np.float32)

        grp4 = x[b, 0:8192:4, :]                            # [2048, C]
        r0c = 512 * q
        cpre = np.concatenate(
            [grp4[0:r0c], np.zeros((XC - r0c, C), np.float32)], axis=0)

        slab = np.concatenate([quarter, bpre, cpre], axis=0)  # [XW, C]
        xt = np.ascontiguousarray(
            slab.T.reshape(2, 128, XW).transpose(1, 0, 2).reshape(128, 2 * XW)
        ).astype(bf16)

        bias = np.zeros((128, 4), np.float32)
        bias[:, 0] = 0.0 if r0 == 1024 else NEG
        for g in range(3):
            bias[:, 1 + g] = 0.0 if g < q else NEG

        in_maps.append({"x": xt, "w": w_t, "bias": bias})
    return in_maps


def kernel(x, Wq, Wk, Wv):
    from concourse.bass_utils import run_bass_kernel_spmd

    nc = _get_program()
    in_maps = make_in_maps(x, Wq, Wk, Wv)
    res = run_bass_kernel_spmd(nc, in_maps, core_ids=list(range(NCORES)))
    out = np.empty((B, N, C), np.float32)
    for d in range(NCORES):
        b, q = divmod(d, 4)
        out[b, 2048 * q:2048 * (q + 1), :] = res.results[d]["out"]
    return out


# revision 3
# speedup vs baseline: 1.1161x; 1.0015x over previous
"""Dilated self-attention TRN2 Bass kernel, v2 (bf16, exact causal shapes).

Problem (hardcoded): B=2, N=8192, C=256, WS=[2048,4096,8192], RS=[1,2,4],
HEAD_IDX=0 -> G=7 groups of s=2048 rows each.

Sharding: core d = (b=d//4, q=d%4) owns output positions [2048q, 2048(q+1))
of batch b.  Jobs per core (all bf16 matmuls, 1 cycle/row at any width):
  C: 4 q-tiles (slots 12..15) of the r=4 group, prefix 12 kt (data-masked)
  B: 8 q-tiles (slots 8..15) of the r=2 half, prefix 8 kt (data-masked)
  A: 16 q-tiles full causal over own quarter
Layout tricks vs v1:
  - One bf16 x slab [128,2,4608] = [A 2048 | Bpre 1024 | Cpre 1536]; B/C
    diag K/Q are strided views into the A region (no re-projection).
  - V tiles for B/C diag are partition-gathered from A's V tiles via
    SBUF->SBUF DMA (stride 2/4) instead of re-projecting.
  - Exact per-q-tile causal prefixes (no 512-block rounding).
  - Scatter-add runs inside the output PSUM accumulation group: placement
    matmuls for B/C land before A's U matmuls; finalize reads PSUM once.
  - Emission order starts the C job while A/B projections still run.
"""

import numpy as np

B, N, C = 2, 8192, 256
S = 2048
NCORES = 8
SCALE = 0.0625
NEG = -1.0e9

XA, XB, XC = 2048, 1024, 1536          # slab section widths
XW = XA + XB + XC                      # 4608

_PROG = None


def _build_program():
    import concourse.mybir as mybir
    import concourse.tile as tile
    from concourse import bacc

    F32 = mybir.dt.float32
    BF16 = mybir.dt.bfloat16
    Exp = mybir.ActivationFunctionType.Exp

    nc = bacc.Bacc("TRN2", target_bir_lowering=False, debug=False,
                   num_devices=NCORES)

    x_d = nc.dram_tensor("x", [128, 2 * XW], BF16, kind="ExternalInput")
    w_d = nc.dram_tensor("w", [128, 2 * 3 * C], BF16, kind="ExternalInput")
    bias_d = nc.dram_tensor("bias", [128, 4], F32, kind="ExternalInput")
    out_d = nc.dram_tensor("out", [S, C], F32, kind="ExternalOutput")

    with tile.TileContext(nc) as tc:
        with (
            tc.tile_pool(name="const", bufs=1) as cpool,
            tc.tile_pool(name="xsb", bufs=1) as xpool,
            tc.tile_pool(name="kqt", bufs=1) as kqpool,
            tc.tile_pool(name="vext", bufs=24) as vpool,
            tc.tile_pool(name="probs", bufs=6) as ppool,
            tc.tile_pool(name="probs2", bufs=6) as ppool2,
            tc.tile_pool(name="stage", bufs=12) as spool,
            tc.tile_pool(name="fin", bufs=4) as fpool,
            tc.tile_pool(name="ps_s", bufs=3, space="PSUM") as ps_scores,
            tc.tile_pool(name="ps_u", bufs=2, space="PSUM") as ps_small,
        ):
            # ---- weights + early x chunks first (PE warmup) ----
            w_sb = cpool.tile([128, 2, 3 * C], BF16, tag="w")
            nc.sync.dma_start(w_sb[:], w_d.rearrange("p (c k) -> p c k", c=2))

            xb = xpool.tile([128, 2, XW], BF16, tag="xb")
            x_r = x_d.rearrange("p (c k) -> p c k", c=2)
            CH = 512
            CHUNK_ORDER = [0, 1, 6, 7, 8, 2, 3, 4, 5]
            for i, ch in enumerate(CHUNK_ORDER):
                eng = nc.sync if i % 2 == 0 else nc.scalar
                eng.dma_start(xb[:, :, CH * ch:CH * (ch + 1)],
                              x_r[:, :, CH * ch:CH * (ch + 1)])
            bias_t = cpool.tile([128, 4], F32, tag="bias")
            nc.sync.dma_start(bias_t[:], bias_d[:])

            def wap(name, ci, co):
                i = {"q": 0, "k": 1, "v": 2}[name]
                return w_sb[:, ci, 256 * i + 128 * co:256 * i + 128 * (co + 1)]

            # ---- constants ----
            ones_t = cpool.tile([128, 128], F32, tag="ones")
            nc.gpsimd.memset(ones_t[:], 1.0)
            # identity (stationary) and -1e9 strict-upper (moving): adding
            # ident.T @ trineg to a diag score tile applies the causal mask
            # inside the PSUM accumulation group, PE-side.
            id_f = cpool.tile([128, 128], F32, tag="id_f")
            nc.gpsimd.affine_select(
                out=id_f[:], in_=ones_t[:],
                compare_op=mybir.AluOpType.is_equal,
                fill=0.0, base=0, pattern=[[1, 128]], channel_multiplier=-1)
            ident = cpool.tile([128, 128], BF16, tag="ident")
            nc.vector.tensor_copy(ident[:], id_f[:])
            neg_t = cpool.tile([128, 128], F32, tag="negs")
            nc.gpsimd.memset(neg_t[:], NEG)
            tri_f = cpool.tile([128, 128], F32, tag="tri_f")
            # keep NEG where p > f (mask keys k > query q), 0 elsewhere
            nc.gpsimd.affine_select(
                out=tri_f[:], in_=neg_t[:],
                compare_op=mybir.AluOpType.is_ge,
                fill=0.0, base=-1, pattern=[[-1, 128]], channel_multiplier=1)
            trineg = cpool.tile([128, 128], BF16, tag="trineg")
            nc.vector.tensor_copy(trineg[:], tri_f[:])

            pmats = {}
            for stride, u in [(2, 0), (2, 1), (4, 0), (4, 1), (4, 2), (4, 3)]:
                pf = cpool.tile([128, 128], F32, tag="fscratch",
                                name=f"pmf{stride}_{u}")
                nc.gpsimd.affine_select(
                    out=pf[:], in_=ones_t[:],
                    compare_op=mybir.AluOpType.is_equal,
                    fill=0.0, base=128 * u,
                    pattern=[[1, 128]], channel_multiplier=-stride)
                pm = cpool.tile([128, 128], BF16, tag=f"pm{stride}_{u}")
                nc.vector.tensor_copy(pm[:], pf[:])
                pmats[stride, u] = pm

            # ---- projection helpers (emitted lazily, in job-driven order) --
            ktb = kqpool.tile([128, 2, XW], BF16, tag="ktb")
            qtb = kqpool.tile([128, 2, XA], BF16, tag="qtb")
            _cp_flip = [0]

            def copy_to(dst, src):
                _cp_flip[0] ^= 1
                if _cp_flip[0]:
                    nc.scalar.copy(dst, src)
                else:
                    nc.vector.tensor_copy(dst, src)

            def kproj(ch):
                c0 = CH * ch
                for co in range(2):
                    ps = ps_small.tile([128, 512], F32, tag="proj", bufs=3)
                    for ci in range(2):
                        nc.tensor.matmul(
                            ps[:], wap("k", ci, co), xb[:, ci, c0:c0 + CH],
                            start=(ci == 0), stop=(ci == 1))
                    copy_to(ktb[:, co, c0:c0 + CH], ps[:])

            def qproj(ch):
                c0 = CH * ch
                for co in range(2):
                    ps = ps_small.tile([128, 512], F32, tag="proj", bufs=3)
                    for ci in range(2):
                        nc.tensor.matmul(
                            ps[:], wap("q", ci, co), xb[:, ci, c0:c0 + CH],
                            start=(ci == 0), stop=(ci == 1))
                    copy_to(qtb[:, co, c0:c0 + CH], ps[:])

            def vproj_pair(src_c0, name):
                """project x cols [src_c0, src_c0+256) -> one [128,2,258]"""
                ps = ps_small.tile([128, 512], F32, tag="proj", bufs=3,
                                   name=f"psv{name}")
                for half in range(2):
                    for ci in range(2):
                        nc.tensor.matmul(
                            ps[:, 256 * half:256 * (half + 1)],
                            xb[:, ci,
                               src_c0 + 128 * half:src_c0 + 128 * (half + 1)],
                            w_sb[:, ci, 512:768],
                            start=(ci == 0), stop=(ci == 1))
                v = vpool.tile([128, 2, 258], BF16, tag="vp", name=f"vp{name}")
                copy_to(v[:, :, 0:256],
                        ps.rearrange("p (h c) -> p h c", h=2))
                nc.vector.memset(v[:, :, 256:257], 1.0)
                nc.vector.memset(v[:, :, 257:258], 0.0)
                return v

            vA = [None] * 8

            def vAp(j):
                if vA[j] is None:
                    vA[j] = vproj_pair(256 * j, f"A{j}")
                return vA[j]

            def vBd_gather(j):
                """B diag pair j rows = A rows [512j, 512j+512) step 2."""
                v = vpool.tile([128, 2, 258], BF16, tag="vp", name=f"vBd{j}")
                for half in range(2):
                    for s in range(2):
                        src = vAp(2 * j + half)
                        nc.sync.dma_start(
                            v[64 * s:64 * (s + 1), half, :],
                            src[0:128:2, s, :])
                return v

            def vCd_gather(j):
                """C diag pair j rows = A rows [1024j, 1024j+1024) step 4."""
                v = vpool.tile([128, 2, 258], BF16, tag="vp", name=f"vCd{j}")
                for half in range(2):
                    base = 512 * (2 * j + half)
                    for s in range(4):
                        src = vAp((base + 128 * s) // 256)
                        sh = ((base + 128 * s) // 128) % 2
                        nc.sync.dma_start(
                            v[32 * s:32 * (s + 1), half, :],
                            src[0:128:4, sh, :])
                return v

            jobs = {
                "C": dict(nq=4, P=12, stride=4, kpre=XA + XB, dstep=4),
                "B": dict(nq=8, P=8, stride=2, kpre=XA, dstep=2),
                "A": dict(nq=16, P=0, stride=1, kpre=None, dstep=1),
            }
            vslabs = {}

            def kstat(job, ci, kt):
                j = jobs[job]
                if kt < j["P"]:
                    c0 = j["kpre"] + 128 * kt
                    return ktb[:, ci, c0:c0 + 128]
                d = kt - j["P"]
                st = j["dstep"]
                return ktb[:, ci, 128 * st * d:128 * st * (d + 1):st]

            def qmov(job, ci, tau):
                st = jobs[job]["dstep"]
                return qtb[:, ci, 128 * st * tau:128 * st * (tau + 1):st]

            def vmov(job, kt):
                j = vslabs[job][kt // 2]
                return j[:, kt % 2, :]

            st_tiles = {}

            def do_qtile(job, tau, ups):
                """scores+exp+U for q-tile tau of job; U accumulated into
                psum tile `ups` (A: group already started by placements)."""
                j = jobs[job]
                P = j["P"]
                nkt = P + tau + 1
                pbs = []            # (pb_tile, kt0, nkt_in_group)
                for gi, g0 in enumerate(range(0, nkt, 4)):
                    gn = min(4, nkt - g0)
                    ps = ps_scores.tile([128, 512], F32, tag="scores")
                    for i in range(gn):
                        kt = g0 + i
                        diag = kt == nkt - 1
                        reg = ps[:, 128 * i:128 * (i + 1)]
                        for ci in range(2):
                            nc.tensor.matmul(
                                reg, kstat(job, ci, kt), qmov(job, ci, tau),
                                start=(ci == 0), stop=(ci == 1 and not diag))
                        if diag:
                            nc.tensor.matmul(reg, ident[:], trineg[:],
                                             start=False, stop=True)
                    # one exp per 4-kt group (P is a multiple of 4, so a
                    # group is uniformly prefix or diag -> single bias)
                    if g0 < P:
                        b0 = 0 if job == "B" else 1 + g0 // 4
                        bias_ap = bias_t[:, b0:b0 + 1]
                    else:
                        bias_ap = 0.0
                    pb = (ppool if gi % 2 == 0 else ppool2).tile(
                        [128, 512], BF16, tag="pb")
                    pbs.append((pb, g0, gn))
                    nc.scalar.activation(pb[:, 0:128 * gn], ps[:, 0:128 * gn],
                                         Exp, bias=bias_ap, scale=SCALE)
                for pb, g0, gn in pbs:
                    for i in range(gn):
                        kt = g0 + i
                        nc.tensor.matmul(
                            ups[:], pb[:, 128 * i:128 * (i + 1)],
                            vmov(job, kt),
                            start=(kt == 0 and job != "A"),
                            stop=(kt == nkt - 1))

            def run_shared(job, tau):
                ups = ps_small.tile([128, 258], F32, tag="u",
                                    name=f"u{job}{tau}")
                do_qtile(job, tau, ups)
                st = spool.tile([128, 258], BF16, tag="st",
                                name=f"st{job}{tau}")
                nc.vector.tensor_copy(st[:], ups[:])
                st_tiles[job, tau] = st

            # ================= emission schedule =================
            # projections needed for C first, C starts ASAP, then B, then A.
            kproj(0); qproj(0)
            vAp(0); vAp(1)
            kproj(1); qproj(1)
            vAp(2); vAp(3)
            for ch in (6, 7, 8):
                kproj(ch)
            vCpre = [vproj_pair(XA + XB + 256 * j, f"Cp{j}") for j in range(6)]
            vslabC = vCpre + [vCd_gather(0)]
            vslabs["C"] = vslabC           # vCd[1] appended below

            run_shared("C", 0)
            kproj(2); qproj(2)
            vAp(4); vAp(5)
            run_shared("C", 1)
            kproj(3); qproj(3)
            vAp(6); vAp(7)
            vslabC.append(vCd_gather(1))
            run_shared("C", 2)
            kproj(4); kproj(5)
            vBpre = [vproj_pair(XA + 256 * j, f"Bp{j}") for j in range(4)]
            run_shared("C", 3)
            vBd = [vBd_gather(j) for j in range(4)]
            vslabs["B"] = vBpre + vBd
            vslabs["A"] = vA

            # ---- B job interleaved with A (A tile t needs st_B[t//2]) ----
            fin = [fpool.tile([128, 2, 256], F32, bufs=1, tag=f"fing{g}",
                              name=f"fing{g}") for g in range(8)]
            out_r = out_d.rearrange("(g t p) c -> g p t c", p=128, t=2)

            def run_A(t):
                ups = ps_small.tile([128, 258], F32, tag="u", name=f"uA{t}")
                nc.tensor.matmul(ups[:], pmats[4, t % 4][:],
                                 st_tiles["C", t // 4][:],
                                 start=True, stop=False)
                nc.tensor.matmul(ups[:], pmats[2, t % 2][:],
                                 st_tiles["B", t // 2][:],
                                 start=False, stop=False)
                do_qtile("A", t, ups)
                g, ti = divmod(t, 2)
                rec = fpool.tile([128, 1], F32, tag="rec")
                nc.vector.reciprocal(rec[:], ups[:, 256:257])
                nc.vector.tensor_scalar_mul(fin[g][:, ti, :],
                                            ups[:, 0:256], rec[:])
                if ti == 1:
                    eng = nc.sync if g % 2 == 0 else nc.scalar
                    eng.dma_start(out_r[g], fin[g][:])

            for tau in range(8):
                run_shared("B", tau)
                run_A(2 * tau)
                run_A(2 * tau + 1)

    nc.compile()
    return nc


def _get_program():
    global _PROG
    if _PROG is None:
        _PROG = _build_program()
    return _PROG


def make_in_maps(x, Wq, Wk, Wv):
    """Host-side sharding: gather / transpose / zero-pad / dtype cast only."""
    import ml_dtypes
    bf16 = np.dtype(ml_dtypes.bfloat16)
    x = np.asarray(x, dtype=np.float32)
    w_all = np.concatenate([np.asarray(Wq, np.float32),
                            np.asarray(Wk, np.float32),
                            np.asarray(Wv, np.float32)], axis=1)  # [C, 3C]
    w_t = np.ascontiguousarray(
        w_all.reshape(2, 128, 3 * C).transpose(1, 0, 2).reshape(128, 6 * C)
    ).astype(bf16)

    in_maps = []
    for d in range(NCORES):
        b, q = divmod(d, 4)
        quarter = x[b, 2048 * q:2048 * (q + 1), :]          # [2048, C]

        seg = 0 if q < 2 else 4096
        grp2 = x[b, seg:seg + 4096:2, :]                    # [2048, C]
        r0 = 1024 * (q % 2)
        bpre = grp2[0:1024] if r0 == 1024 else np.zeros((XB, C), np.float32)

        grp4 = x[b, 0:8192:4, :]                            # [2048, C]
        r0c = 512 * q
        cpre = np.concatenate(
            [grp4[0:r0c], np.zeros((XC - r0c, C), np.float32)], axis=0)

        slab = np.concatenate([quarter, bpre, cpre], axis=0)  # [XW, C]
        xt = np.ascontiguousarray(
            slab.T.reshape(2, 128, XW).transpose(1, 0, 2).reshape(128, 2 * XW)
        ).astype(bf16)

        bias = np.zeros((128, 4), np.float32)
        bias[:, 0] = 0.0 if r0 == 1024 else NEG
        for g in range(3):
            bias[:, 1 + g] = 0.0 if g < q else NEG

        in_maps.append({"x": xt, "w": w_t, "bias": bias})
    return in_maps


def kernel(x, Wq, Wk, Wv):
    from concourse.bass_utils import run_bass_kernel_spmd

    nc = _get_program()
    in_maps = make_in_maps(x, Wq, Wk, Wv)
    res = run_bass_kernel_spmd(nc, in_maps, core_ids=list(range(NCORES)))
    out = np.empty((B, N, C), np.float32)
    for d in range(NCORES):
        b, q = divmod(d, 4)
        out[b, 2048 * q:2048 * (q + 1), :] = res.results[d]["out"]
    return out


# revision 5
# speedup vs baseline: 1.3099x; 1.1736x over previous
"""Dilated self-attention TRN2 Bass kernel, v2 (bf16, exact causal shapes).

Problem (hardcoded): B=2, N=8192, C=256, WS=[2048,4096,8192], RS=[1,2,4],
HEAD_IDX=0 -> G=7 groups of s=2048 rows each.

Sharding: core d = (b=d//4, q=d%4) owns output positions [2048q, 2048(q+1))
of batch b.  Jobs per core (all bf16 matmuls, 1 cycle/row at any width):
  C: 4 q-tiles (slots 12..15) of the r=4 group, prefix 12 kt (data-masked)
  B: 8 q-tiles (slots 8..15) of the r=2 half, prefix 8 kt (data-masked)
  A: 16 q-tiles full causal over own quarter
Layout tricks vs v1:
  - One bf16 x slab [128,2,4608] = [A 2048 | Bpre 1024 | Cpre 1536]; B/C
    diag K/Q are strided views into the A region (no re-projection).
  - V tiles for B/C diag are partition-gathered from A's V tiles via
    SBUF->SBUF DMA (stride 2/4) instead of re-projecting.
  - Exact per-q-tile causal prefixes (no 512-block rounding).
  - Scatter-add runs inside the output PSUM accumulation group: placement
    matmuls for B/C land before A's U matmuls; finalize reads PSUM once.
  - Emission order starts the C job while A/B projections still run.
"""

import numpy as np

B, N, C = 2, 8192, 256
S = 2048
NCORES = 8
SCALE = 0.0625
NEG = -1.0e9

XA, XB, XC = 2048, 1024, 1536          # slab section widths
XW = XA + XB + XC                      # 4608

_PROG = None


def _build_program():
    import concourse.mybir as mybir
    import concourse.tile as tile
    from concourse import bacc

    F32 = mybir.dt.float32
    BF16 = mybir.dt.bfloat16
    FP8 = mybir.dt.float8e4
    DR = mybir.MatmulPerfMode.DoubleRow
    Exp = mybir.ActivationFunctionType.Exp

    nc = bacc.Bacc("TRN2", target_bir_lowering=False, debug=False,
                   num_devices=NCORES)

    x_d = nc.dram_tensor("x", [128, 2 * XW], BF16, kind="ExternalInput")
    w_d = nc.dram_tensor("w", [128, 2 * 3 * C], BF16, kind="ExternalInput")
    bias_d = nc.dram_tensor("bias", [128, 5], F32, kind="ExternalInput")
    out_d = nc.dram_tensor("out", [S, C], F32, kind="ExternalOutput")

    with tile.TileContext(nc) as tc:
        with (
            tc.tile_pool(name="const", bufs=1) as cpool,
            tc.tile_pool(name="xsb", bufs=1) as xpool,
            tc.tile_pool(name="kqt", bufs=1) as kqpool,
            tc.tile_pool(name="vext", bufs=32) as vpool,
            tc.tile_pool(name="probs", bufs=6) as ppool,
            tc.tile_pool(name="probs2", bufs=6) as ppool2,
            tc.tile_pool(name="probs8", bufs=6) as ppool8,
            tc.tile_pool(name="stage", bufs=12) as spool,
            tc.tile_pool(name="fin", bufs=4) as fpool,
            tc.tile_pool(name="ps_s", bufs=3, space="PSUM") as ps_scores,
            tc.tile_pool(name="ps_u", bufs=2, space="PSUM") as ps_small,
        ):
            # ---- weights + early x chunks first (PE warmup) ----
            w_sb = cpool.tile([128, 2, 3 * C], BF16, tag="w")
            nc.sync.dma_start(w_sb[:], w_d.rearrange("p (c k) -> p c k", c=2))

            xb = xpool.tile([128, 2, XW], BF16, tag="xb")
            x_r = x_d.rearrange("p (c k) -> p c k", c=2)
            CH = 512
            CHUNK_ORDER = [0, 1, 6, 7, 8, 2, 3, 4, 5]
            for i, ch in enumerate(CHUNK_ORDER):
                eng = nc.sync if i % 2 == 0 else nc.scalar
                eng.dma_start(xb[:, :, CH * ch:CH * (ch + 1)],
                              x_r[:, :, CH * ch:CH * (ch + 1)])
            bias_t = cpool.tile([128, 5], F32, tag="bias")
            nc.sync.dma_start(bias_t[:], bias_d[:])

            def wap(name, ci, co):
                i = {"q": 0, "k": 1, "v": 2}[name]
                return w_sb[:, ci, 256 * i + 128 * co:256 * i + 128 * (co + 1)]

            # ---- constants ----
            ones_t = cpool.tile([128, 128], F32, tag="ones")
            nc.gpsimd.memset(ones_t[:], 1.0)
            # identity (stationary) and -1e9 strict-upper (moving): adding
            # ident.T @ trineg to a diag score tile applies the causal mask
            # inside the PSUM accumulation group, PE-side.
            id_f = cpool.tile([128, 128], F32, tag="id_f")
            nc.gpsimd.affine_select(
                out=id_f[:], in_=ones_t[:],
                compare_op=mybir.AluOpType.is_equal,
                fill=0.0, base=0, pattern=[[1, 128]], channel_multiplier=-1)
            ident = cpool.tile([128, 128], BF16, tag="ident")
            nc.vector.tensor_copy(ident[:], id_f[:])
            neg_t = cpool.tile([128, 128], F32, tag="negs")
            nc.gpsimd.memset(neg_t[:], NEG)
            tri_f = cpool.tile([128, 128], F32, tag="tri_f")
            # keep NEG where p > f (mask keys k > query q), 0 elsewhere
            nc.gpsimd.affine_select(
                out=tri_f[:], in_=neg_t[:],
                compare_op=mybir.AluOpType.is_ge,
                fill=0.0, base=-1, pattern=[[-1, 128]], channel_multiplier=1)
            trineg = cpool.tile([128, 128], BF16, tag="trineg")
            nc.vector.tensor_copy(trineg[:], tri_f[:])

            pmats = {}
            for stride, u in [(2, 0), (2, 1), (4, 0), (4, 1), (4, 2), (4, 3)]:
                pf = cpool.tile([128, 128], F32, tag="fscratch",
                                name=f"pmf{stride}_{u}")
                nc.gpsimd.affine_select(
                    out=pf[:], in_=ones_t[:],
                    compare_op=mybir.AluOpType.is_equal,
                    fill=0.0, base=128 * u,
                    pattern=[[1, 128]], channel_multiplier=-stride)
                pm = cpool.tile([128, 128], BF16, tag=f"pm{stride}_{u}")
                nc.vector.tensor_copy(pm[:], pf[:])
                pmats[stride, u] = pm

            # ---- projection helpers (emitted lazily, in job-driven order) --
            ktb = kqpool.tile([128, 2, XA], BF16, tag="ktb")
            qtb = kqpool.tile([128, 2, XA], BF16, tag="qtb")
            kt8 = kqpool.tile([128, 2, XB + XC], FP8, tag="kt8")
            qt8 = kqpool.tile([128, 2, 1536], FP8, tag="qt8")
            kt8a = kqpool.tile([128, 2, XA], FP8, tag="kt8a")
            qt8a = kqpool.tile([128, 2, XA], FP8, tag="qt8a")
            _cp_flip = [0]

            def copy_to(dst, src):
                _cp_flip[0] ^= 1
                if _cp_flip[0]:
                    nc.scalar.copy(dst, src)
                else:
                    nc.vector.tensor_copy(dst, src)

            def kproj(ch):
                c0 = CH * ch
                for co in range(2):
                    ps = ps_small.tile([128, 512], F32, tag="proj", bufs=3)
                    for ci in range(2):
                        nc.tensor.matmul(
                            ps[:], wap("k", ci, co), xb[:, ci, c0:c0 + CH],
                            start=(ci == 0), stop=(ci == 1))
                    if c0 >= XA:
                        nc.vector.tensor_copy(
                            kt8[:, co, c0 - XA:c0 - XA + CH], ps[:])
                    else:
                        copy_to(ktb[:, co, c0:c0 + CH], ps[:])

            def qproj(ch):
                c0 = CH * ch
                for co in range(2):
                    ps = ps_small.tile([128, 512], F32, tag="proj", bufs=3)
                    for ci in range(2):
                        nc.tensor.matmul(
                            ps[:], wap("q", ci, co), xb[:, ci, c0:c0 + CH],
                            start=(ci == 0), stop=(ci == 1))
                    copy_to(qtb[:, co, c0:c0 + CH], ps[:])
                    nc.vector.tensor_copy(
                        qt8[:, co, c0 // 2:c0 // 2 + 256], ps[:, 0:512:2])
                    nc.vector.tensor_copy(
                        qt8[:, co, 1024 + c0 // 4:1024 + c0 // 4 + 128],
                        ps[:, 0:512:4])

            def vproj_pair(src_c0, name, dt=BF16):
                """project x cols [src_c0, src_c0+256) -> one [128,2,258]"""
                ps = ps_small.tile([128, 512], F32, tag="proj", bufs=3,
                                   name=f"psv{name}")
                for half in range(2):
                    for ci in range(2):
                        nc.tensor.matmul(
                            ps[:, 256 * half:256 * (half + 1)],
                            xb[:, ci,
                               src_c0 + 128 * half:src_c0 + 128 * (half + 1)],
                            w_sb[:, ci, 512:768],
                            start=(ci == 0), stop=(ci == 1))
                v = vpool.tile([128, 2, 258], dt, tag="vp", name=f"vp{name}")
                if dt is BF16:
                    copy_to(v[:, :, 0:256],
                            ps.rearrange("p (h c) -> p h c", h=2))
                else:
                    nc.vector.tensor_copy(v[:, :, 0:256],
                                          ps.rearrange("p (h c) -> p h c", h=2))
                nc.vector.memset(v[:, :, 256:257], 1.0)
                nc.vector.memset(v[:, :, 257:258], 0.0)
                return v

            vA = [None] * 8
            vA8 = [None] * 8

            def vAp(j):
                if vA[j] is None:
                    vA[j] = vproj_pair(256 * j, f"A{j}")
                return vA[j]

            def vBd_gather(j):
                """B diag pair j rows = A rows [512j, 512j+512) step 2."""
                v = vpool.tile([128, 2, 258], BF16, tag="vp", name=f"vBd{j}")
                for half in range(2):
                    for s in range(2):
                        src = vAp(2 * j + half)
                        nc.sync.dma_start(
                            v[64 * s:64 * (s + 1), half, :],
                            src[0:128:2, s, :])
                return v

            def vCd_gather(j):
                """C diag pair j rows = A rows [1024j, 1024j+1024) step 4."""
                v = vpool.tile([128, 2, 258], BF16, tag="vp", name=f"vCd{j}")
                for half in range(2):
                    base = 512 * (2 * j + half)
                    for s in range(4):
                        src = vAp((base + 128 * s) // 256)
                        sh = ((base + 128 * s) // 128) % 2
                        nc.sync.dma_start(
                            v[32 * s:32 * (s + 1), half, :],
                            src[0:128:4, sh, :])
                return v

            jobs = {
                "C": dict(nq=4, P=12, stride=4, kpre=XA + XB, dstep=4),
                "B": dict(nq=8, P=8, stride=2, kpre=XA, dstep=2),
                "A": dict(nq=16, P=0, stride=1, kpre=None, dstep=1),
            }
            vslabs = {}

            def kstat(job, ci, kt):
                j = jobs[job]
                if kt < j["P"]:
                    c0 = j["kpre"] + 128 * kt
                    return ktb[:, ci, c0:c0 + 128]
                d = kt - j["P"]
                st = j["dstep"]
                return ktb[:, ci, 128 * st * d:128 * st * (d + 1):st]

            def qmov(job, ci, tau):
                st = jobs[job]["dstep"]
                return qtb[:, ci, 128 * st * tau:128 * st * (tau + 1):st]

            def kstat8(job, kt):
                if job == "A":
                    return kt8a[:, :, 128 * kt:128 * (kt + 1)]
                off = 0 if job == "B" else XB
                return kt8[:, :, off + 128 * kt:off + 128 * (kt + 1)]

            def qmov8(job, tau):
                if job == "A":
                    return qt8a[:, :, 128 * tau:128 * (tau + 1)]
                off = 0 if job == "B" else 1024
                return qt8[:, :, off + 128 * tau:off + 128 * (tau + 1)]

            def vmov(job, kt):
                j = vslabs[job][kt // 2]
                return j[:, kt % 2, :]

            st_tiles = {}

            def do_qtile(job, tau, ups):
                """scores+exp+U for q-tile tau of job; U accumulated into
                psum tile `ups` (A: group already started by placements)."""
                j = jobs[job]
                P = j["P"]
                nkt = P + tau + 1
                pbs = []            # (pb_tile, kt0, nkt_in_group, is8)
                for gi, g0 in enumerate(range(0, nkt, 4)):
                    gn = min(4, nkt - g0)
                    # groups fully inside the prefix use the fp8 DoubleRow
                    # path (long-range diffuse attention tolerates fp8)
                    if job == "A":
                        is8 = g0 + gn - 1 <= tau - 5
                    else:
                        is8 = g0 + gn <= P
                    ps = ps_scores.tile([128, 512], F32, tag="scores")
                    if is8:
                        for i in range(gn):
                            kt = g0 + i
                            nc.tensor.matmul(
                                ps[:, 128 * i:128 * (i + 1)],
                                kstat8(job, kt), qmov8(job, tau),
                                start=True, stop=True, perf_mode=DR)
                        if job == "A":
                            bias_ap = bias_t[:, 4:5]
                        else:
                            b0 = 0 if job == "B" else 1 + g0 // 4
                            bias_ap = bias_t[:, b0:b0 + 1]
                        pb = ppool8.tile([128, 512], FP8, tag="pb8")
                    else:
                        for i in range(gn):
                            kt = g0 + i
                            diag = kt == nkt - 1
                            reg = ps[:, 128 * i:128 * (i + 1)]
                            for ci in range(2):
                                nc.tensor.matmul(
                                    reg, kstat(job, ci, kt),
                                    qmov(job, ci, tau),
                                    start=(ci == 0),
                                    stop=(ci == 1 and not diag))
                            if diag:
                                nc.tensor.matmul(reg, ident[:], trineg[:],
                                                 start=False, stop=True)
                        if g0 < P:
                            b0 = 0 if job == "B" else 1 + g0 // 4
                            bias_ap = bias_t[:, b0:b0 + 1]
                        else:
                            bias_ap = bias_t[:, 4:5]
                        pb = (ppool if gi % 2 == 0 else ppool2).tile(
                            [128, 512], BF16, tag="pb")
                    pbs.append((pb, g0, gn, is8))
                    nc.scalar.activation(pb[:, 0:128 * gn], ps[:, 0:128 * gn],
                                         Exp, bias=bias_ap, scale=SCALE)
                for pb, g0, gn, is8 in pbs:
                    if is8:
                        for jj in range(gn // 2):
                            kt = g0 + 2 * jj
                            nc.tensor.matmul(
                                ups[:],
                                pb[:, 256 * jj:256 * (jj + 1)].rearrange(
                                    "p (two f) -> p two f", two=2),
                                (vA8 if job == "A"
                                 else vslabs[job])[kt // 2][:],
                                start=(kt == 0 and job != "A"),
                                stop=False, perf_mode=DR)
                    else:
                        for i in range(gn):
                            kt = g0 + i
                            nc.tensor.matmul(
                                ups[:], pb[:, 128 * i:128 * (i + 1)],
                                vmov(job, kt),
                                start=(kt == 0 and job != "A"),
                                stop=(kt == nkt - 1))

            def run_shared(job, tau):
                ups = ps_small.tile([128, 258], F32, tag="u",
                                    name=f"u{job}{tau}")[:]
                do_qtile(job, tau, ups)
                st = spool.tile([128, 258], BF16, tag="st",
                                name=f"st{job}{tau}")
                nc.vector.tensor_copy(st[:], ups[:])
                st_tiles[job, tau] = st

            # ================= emission schedule =================
            # projections needed for C first, C starts ASAP, then B, then A.
            kproj(0); qproj(0)
            vAp(0); vAp(1)
            kproj(1); qproj(1)
            vAp(2); vAp(3)
            for ch in (6, 7, 8):
                kproj(ch)
            vCpre = [vproj_pair(XA + XB + 256 * j, f"Cp{j}", FP8) for j in range(6)]
            vslabC = vCpre + [vCd_gather(0)]
            vslabs["C"] = vslabC           # vCd[1] appended below

            run_shared("C", 0)
            kproj(2); qproj(2)
            vAp(4); vAp(5)
            run_shared("C", 1)
            kproj(3); qproj(3)
            vAp(6); vAp(7)
            vslabC.append(vCd_gather(1))
            run_shared("C", 2)
            kproj(4); kproj(5)
            vBpre = [vproj_pair(XA + 256 * j, f"Bp{j}", FP8) for j in range(4)]
            run_shared("C", 3)
            vBd = [vBd_gather(j) for j in range(4)]
            vslabs["B"] = vBpre + vBd
            vslabs["A"] = vA

            # ---- B job interleaved with A (A tile t needs st_B[t//2]) ----
            fin = [fpool.tile([128, 2, 256], F32, bufs=1, tag=f"fing{g}",
                              name=f"fing{g}") for g in range(8)]
            out_r = out_d.rearrange("(g t p) c -> g p t c", p=128, t=2)

            def run_A(t):
                ups = ps_small.tile([128, 258], F32, tag="u",
                                    name=f"uA{t}")[:]
                nc.tensor.matmul(ups, pmats[4, t % 4][:],
                                 st_tiles["C", t // 4][:],
                                 start=True, stop=False)
                nc.tensor.matmul(ups, pmats[2, t % 2][:],
                                 st_tiles["B", t // 2][:],
                                 start=False, stop=False)
                do_qtile("A", t, ups)
                g, ti = divmod(t, 2)
                rec = fpool.tile([128, 1], F32, tag="rec")
                nc.vector.reciprocal(rec[:], ups[:, 256:257])
                nc.vector.tensor_scalar_mul(fin[g][:, ti, :],
                                            ups[:, 0:256], rec[:])
                if ti == 1:
                    eng = nc.sync if g % 2 == 0 else nc.scalar
                    eng.dma_start(out_r[g], fin[g][:])

            # deferred fp8 twins of A-region K/Q/V (used by far A tiles)
            for ch in range(4):
                for co in range(2):
                    eng = nc.scalar if (ch + co) % 2 else None
                    c0 = CH * ch
                    if eng:
                        eng.copy(kt8a[:, co, c0:c0 + CH],
                                 ktb[:, co, c0:c0 + CH])
                        eng.copy(qt8a[:, co, c0:c0 + CH],
                                 qtb[:, co, c0:c0 + CH])
                    else:
                        nc.vector.tensor_copy(kt8a[:, co, c0:c0 + CH],
                                              ktb[:, co, c0:c0 + CH])
                        nc.vector.tensor_copy(qt8a[:, co, c0:c0 + CH],
                                              qtb[:, co, c0:c0 + CH])
            for j in range(8):
                v8 = vpool.tile([128, 2, 258], FP8, tag="vp", name=f"vA8_{j}")
                nc.vector.tensor_copy(v8[:], vA[j][:])
                vA8[j] = v8

            for tau in range(8):
                run_shared("B", tau)
                run_A(2 * tau)
                run_A(2 * tau + 1)

    nc.compile()
    return nc


def _get_program():
    global _PROG
    if _PROG is None:
        _PROG = _build_program()
    return _PROG


def make_in_maps(x, Wq, Wk, Wv):
    """Host-side sharding: gather / transpose / zero-pad / dtype cast only."""
    import ml_dtypes
    bf16 = np.dtype(ml_dtypes.bfloat16)
    x = np.asarray(x, dtype=np.float32)
    w_all = np.concatenate([np.asarray(Wq, np.float32),
                            np.asarray(Wk, np.float32),
                            np.asarray(Wv, np.float32)], axis=1)  # [C, 3C]
    w_t = np.ascontiguousarray(
        w_all.reshape(2, 128, 3 * C).transpose(1, 0, 2).reshape(128, 6 * C)
    ).astype(bf16)

    in_maps = []
    for d in range(NCORES):
        b, q = divmod(d, 4)
        quarter = x[b, 2048 * q:2048 * (q + 1), :]          # [2048, C]

        seg = 0 if q < 2 else 4096
        grp2 = x[b, seg:seg + 4096:2, :]                    # [2048, C]
        r0 = 1024 * (q % 2)
        bpre = grp2[0:1024] if r0 == 1024 else np.zeros((XB, C), np.float32)

        grp4 = x[b, 0:8192:4, :]                            # [2048, C]
        r0c = 512 * q
        cpre = np.concatenate(
            [grp4[0:r0c], np.zeros((XC - r0c, C), np.float32)], axis=0)

        slab = np.concatenate([quarter, bpre, cpre], axis=0)  # [XW, C]
        xt = np.ascontiguousarray(
            slab.T.reshape(2, 128, XW).transpose(1, 0, 2).reshape(128, 2 * XW)
        ).astype(bf16)

        bias = np.zeros((128, 5), np.float32)
        bias[:, 4] = -2.0
        bias[:, 0] = -2.0 if r0 == 1024 else NEG
        for g in range(3):
            bias[:, 1 + g] = -2.0 if g < q else NEG

        in_maps.append({"x": xt, "w": w_t, "bias": bias})
    return in_maps


def kernel(x, Wq, Wk, Wv):
    from concourse.bass_utils import run_bass_kernel_spmd

    nc = _get_program()
    in_maps = make_in_maps(x, Wq, Wk, Wv)
    res = run_bass_kernel_spmd(nc, in_maps, core_ids=list(range(NCORES)))
    out = np.empty((B, N, C), np.float32)
    for d in range(NCORES):
        b, q = divmod(d, 4)
        out[b, 2048 * q:2048 * (q + 1), :] = res.results[d]["out"]
    return out


# revision 6
# speedup vs baseline: 1.3397x; 1.0227x over previous
"""Dilated self-attention TRN2 Bass kernel, v2 (bf16, exact causal shapes).

Problem (hardcoded): B=2, N=8192, C=256, WS=[2048,4096,8192], RS=[1,2,4],
HEAD_IDX=0 -> G=7 groups of s=2048 rows each.

Sharding: core d = (b=d//4, q=d%4) owns output positions [2048q, 2048(q+1))
of batch b.  Jobs per core (all bf16 matmuls, 1 cycle/row at any width):
  C: 4 q-tiles (slots 12..15) of the r=4 group, prefix 12 kt (data-masked)
  B: 8 q-tiles (slots 8..15) of the r=2 half, prefix 8 kt (data-masked)
  A: 16 q-tiles full causal over own quarter
Layout tricks vs v1:
  - One bf16 x slab [128,2,4608] = [A 2048 | Bpre 1024 | Cpre 1536]; B/C
    diag K/Q are strided views into the A region (no re-projection).
  - V tiles for B/C diag are partition-gathered from A's V tiles via
    SBUF->SBUF DMA (stride 2/4) instead of re-projecting.
  - Exact per-q-tile causal prefixes (no 512-block rounding).
  - Scatter-add runs inside the output PSUM accumulation group: placement
    matmuls for B/C land before A's U matmuls; finalize reads PSUM once.
  - Emission order starts the C job while A/B projections still run.
"""

import numpy as np

B, N, C = 2, 8192, 256
S = 2048
NCORES = 8
SCALE = 0.0625
NEG = -1.0e9

XA, XB, XC = 2048, 1024, 1536          # slab section widths
XW = XA + XB + XC                      # 4608

_PROG = None


def _build_program():
    import concourse.mybir as mybir
    import concourse.tile as tile
    from concourse import bacc

    F32 = mybir.dt.float32
    BF16 = mybir.dt.bfloat16
    FP8 = mybir.dt.float8e4
    DR = mybir.MatmulPerfMode.DoubleRow
    Exp = mybir.ActivationFunctionType.Exp

    nc = bacc.Bacc("TRN2", target_bir_lowering=False, debug=False,
                   num_devices=NCORES)

    x_d = nc.dram_tensor("x", [128, 2 * XW], BF16, kind="ExternalInput")
    w_d = nc.dram_tensor("w", [128, 2 * 3 * C], BF16, kind="ExternalInput")
    bias_d = nc.dram_tensor("bias", [128, 5], F32, kind="ExternalInput")
    out_d = nc.dram_tensor("out", [S, C], F32, kind="ExternalOutput")

    with tile.TileContext(nc) as tc:
        with (
            tc.tile_pool(name="const", bufs=1) as cpool,
            tc.tile_pool(name="xsb", bufs=1) as xpool,
            tc.tile_pool(name="kqt", bufs=1) as kqpool,
            tc.tile_pool(name="vext", bufs=32) as vpool,
            tc.tile_pool(name="probs", bufs=6) as ppool,
            tc.tile_pool(name="probs2", bufs=6) as ppool2,
            tc.tile_pool(name="probs8", bufs=6) as ppool8,
            tc.tile_pool(name="stage", bufs=12) as spool,
            tc.tile_pool(name="fin", bufs=4) as fpool,
            tc.tile_pool(name="ps_s", bufs=3, space="PSUM") as ps_scores,
            tc.tile_pool(name="ps_u", bufs=2, space="PSUM") as ps_small,
        ):
            # ---- weights + early x chunks first (PE warmup) ----
            w_sb = cpool.tile([128, 2, 3 * C], BF16, tag="w")
            nc.sync.dma_start(w_sb[:], w_d.rearrange("p (c k) -> p c k", c=2))

            xb = xpool.tile([128, 2, XW], BF16, tag="xb")
            x_r = x_d.rearrange("p (c k) -> p c k", c=2)
            CH = 512
            CHUNK_ORDER = [0, 1, 6, 7, 8, 2, 3, 4, 5]
            for i, ch in enumerate(CHUNK_ORDER):
                eng = nc.sync if i % 2 == 0 else nc.scalar
                eng.dma_start(xb[:, :, CH * ch:CH * (ch + 1)],
                              x_r[:, :, CH * ch:CH * (ch + 1)])
            bias_t = cpool.tile([128, 5], F32, tag="bias")
            nc.sync.dma_start(bias_t[:], bias_d[:])

            def wap(name, ci, co):
                i = {"q": 0, "k": 1, "v": 2}[name]
                return w_sb[:, ci, 256 * i + 128 * co:256 * i + 128 * (co + 1)]

            # ---- constants ----
            ones_t = cpool.tile([128, 128], F32, tag="ones")
            nc.gpsimd.memset(ones_t[:], 1.0)
            # identity (stationary) and -1e9 strict-upper (moving): adding
            # ident.T @ trineg to a diag score tile applies the causal mask
            # inside the PSUM accumulation group, PE-side.
            id_f = cpool.tile([128, 128], F32, tag="id_f")
            nc.gpsimd.affine_select(
                out=id_f[:], in_=ones_t[:],
                compare_op=mybir.AluOpType.is_equal,
                fill=0.0, base=0, pattern=[[1, 128]], channel_multiplier=-1)
            ident = cpool.tile([128, 128], BF16, tag="ident")
            nc.vector.tensor_copy(ident[:], id_f[:])
            neg_t = cpool.tile([128, 128], F32, tag="negs")
            nc.gpsimd.memset(neg_t[:], NEG)
            tri_f = cpool.tile([128, 128], F32, tag="tri_f")
            # keep NEG where p > f (mask keys k > query q), 0 elsewhere
            nc.gpsimd.affine_select(
                out=tri_f[:], in_=neg_t[:],
                compare_op=mybir.AluOpType.is_ge,
                fill=0.0, base=-1, pattern=[[-1, 128]], channel_multiplier=1)
            trineg = cpool.tile([128, 128], BF16, tag="trineg")
            nc.vector.tensor_copy(trineg[:], tri_f[:])

            pmats = {}
            for stride, u in [(2, 0), (2, 1), (4, 0), (4, 1), (4, 2), (4, 3)]:
                pf = cpool.tile([128, 128], F32, tag="fscratch",
                                name=f"pmf{stride}_{u}")
                nc.gpsimd.affine_select(
                    out=pf[:], in_=ones_t[:],
                    compare_op=mybir.AluOpType.is_equal,
                    fill=0.0, base=128 * u,
                    pattern=[[1, 128]], channel_multiplier=-stride)
                pm = cpool.tile([128, 128], BF16, tag=f"pm{stride}_{u}")
                nc.vector.tensor_copy(pm[:], pf[:])
                pmats[stride, u] = pm

            # ---- projection helpers (emitted lazily, in job-driven order) --
            ktb = kqpool.tile([128, 2, XA], BF16, tag="ktb")
            qtb = kqpool.tile([128, 2, XA], BF16, tag="qtb")
            kt8 = kqpool.tile([128, 2, XB + XC], FP8, tag="kt8")
            qt8 = kqpool.tile([128, 2, 1536], FP8, tag="qt8")
            kt8a = kqpool.tile([128, 2, XA], FP8, tag="kt8a")
            qt8a = kqpool.tile([128, 2, XA], FP8, tag="qt8a")
            _cp_flip = [0]

            def copy_to(dst, src):
                _cp_flip[0] ^= 1
                if _cp_flip[0]:
                    nc.scalar.copy(dst, src)
                else:
                    nc.vector.tensor_copy(dst, src)

            def kproj(ch):
                c0 = CH * ch
                for co in range(2):
                    ps = ps_small.tile([128, 512], F32, tag="proj", bufs=3)
                    for ci in range(2):
                        nc.tensor.matmul(
                            ps[:], wap("k", ci, co), xb[:, ci, c0:c0 + CH],
                            start=(ci == 0), stop=(ci == 1))
                    if c0 >= XA:
                        nc.vector.tensor_copy(
                            kt8[:, co, c0 - XA:c0 - XA + CH], ps[:])
                    else:
                        copy_to(ktb[:, co, c0:c0 + CH], ps[:])

            def qproj(ch):
                c0 = CH * ch
                for co in range(2):
                    ps = ps_small.tile([128, 512], F32, tag="proj", bufs=3)
                    for ci in range(2):
                        nc.tensor.matmul(
                            ps[:], wap("q", ci, co), xb[:, ci, c0:c0 + CH],
                            start=(ci == 0), stop=(ci == 1))
                    copy_to(qtb[:, co, c0:c0 + CH], ps[:])
                    nc.vector.tensor_copy(
                        qt8[:, co, c0 // 2:c0 // 2 + 256], ps[:, 0:512:2])
                    nc.vector.tensor_copy(
                        qt8[:, co, 1024 + c0 // 4:1024 + c0 // 4 + 128],
                        ps[:, 0:512:4])

            def vproj_pair(src_c0, name, dt=BF16):
                """project x cols [src_c0, src_c0+256) -> one [128,2,258]"""
                ps = ps_small.tile([128, 512], F32, tag="proj", bufs=3,
                                   name=f"psv{name}")
                for half in range(2):
                    for ci in range(2):
                        nc.tensor.matmul(
                            ps[:, 256 * half:256 * (half + 1)],
                            xb[:, ci,
                               src_c0 + 128 * half:src_c0 + 128 * (half + 1)],
                            w_sb[:, ci, 512:768],
                            start=(ci == 0), stop=(ci == 1))
                v = vpool.tile([128, 2, 258], dt, tag="vp", name=f"vp{name}")
                if dt is BF16:
                    copy_to(v[:, :, 0:256],
                            ps.rearrange("p (h c) -> p h c", h=2))
                else:
                    nc.vector.tensor_copy(v[:, :, 0:256],
                                          ps.rearrange("p (h c) -> p h c", h=2))
                nc.vector.memset(v[:, :, 256:257], 1.0)
                nc.vector.memset(v[:, :, 257:258], 0.0)
                return v

            vA = [None] * 8
            vA8 = [None] * 8

            def vAp(j):
                if vA[j] is None:
                    vA[j] = vproj_pair(256 * j, f"A{j}")
                return vA[j]

            def vBd_gather(j):
                """B diag pair j rows = A rows [512j, 512j+512) step 2."""
                v = vpool.tile([128, 2, 258], BF16, tag="vp", name=f"vBd{j}")
                for half in range(2):
                    for s in range(2):
                        src = vAp(2 * j + half)
                        nc.sync.dma_start(
                            v[64 * s:64 * (s + 1), half, :],
                            src[0:128:2, s, :])
                return v

            def vCd_gather(j):
                """C diag pair j rows = A rows [1024j, 1024j+1024) step 4."""
                v = vpool.tile([128, 2, 258], BF16, tag="vp", name=f"vCd{j}")
                for half in range(2):
                    base = 512 * (2 * j + half)
                    for s in range(4):
                        src = vAp((base + 128 * s) // 256)
                        sh = ((base + 128 * s) // 128) % 2
                        nc.sync.dma_start(
                            v[32 * s:32 * (s + 1), half, :],
                            src[0:128:4, sh, :])
                return v

            jobs = {
                "C": dict(nq=4, P=12, stride=4, kpre=XA + XB, dstep=4),
                "B": dict(nq=8, P=8, stride=2, kpre=XA, dstep=2),
                "A": dict(nq=16, P=0, stride=1, kpre=None, dstep=1),
            }
            vslabs = {}

            def kstat(job, ci, kt):
                j = jobs[job]
                if kt < j["P"]:
                    c0 = j["kpre"] + 128 * kt
                    return ktb[:, ci, c0:c0 + 128]
                d = kt - j["P"]
                st = j["dstep"]
                return ktb[:, ci, 128 * st * d:128 * st * (d + 1):st]

            def qmov(job, ci, tau):
                st = jobs[job]["dstep"]
                return qtb[:, ci, 128 * st * tau:128 * st * (tau + 1):st]

            def kstat8(job, kt):
                if job == "A":
                    return kt8a[:, :, 128 * kt:128 * (kt + 1)]
                off = 0 if job == "B" else XB
                return kt8[:, :, off + 128 * kt:off + 128 * (kt + 1)]

            def qmov8(job, tau):
                if job == "A":
                    return qt8a[:, :, 128 * tau:128 * (tau + 1)]
                off = 0 if job == "B" else 1024
                return qt8[:, :, off + 128 * tau:off + 128 * (tau + 1)]

            def vmov(job, kt):
                j = vslabs[job][kt // 2]
                return j[:, kt % 2, :]

            st_tiles = {}

            def do_qtile(job, tau, ups):
                """scores+exp+U for q-tile tau of job; U accumulated into
                psum tile `ups` (A: group already started by placements)."""
                j = jobs[job]
                P = j["P"]
                nkt = P + tau + 1
                pbs = []            # (pb_tile, kt0, nkt_in_group, is8)
                for gi, g0 in enumerate(range(0, nkt, 4)):
                    gn = min(4, nkt - g0)
                    # groups fully inside the prefix use the fp8 DoubleRow
                    # path (long-range diffuse attention tolerates fp8)
                    if job == "A":
                        is8 = g0 + gn - 1 <= tau - 5
                    else:
                        is8 = g0 + gn <= P
                    ps = ps_scores.tile([128, 512], F32, tag="scores")
                    if is8:
                        for i in range(gn):
                            kt = g0 + i
                            nc.tensor.matmul(
                                ps[:, 128 * i:128 * (i + 1)],
                                kstat8(job, kt), qmov8(job, tau),
                                start=True, stop=True, perf_mode=DR)
                        if job == "A":
                            bias_ap = bias_t[:, 4:5]
                        else:
                            b0 = 0 if job == "B" else 1 + g0 // 4
                            bias_ap = bias_t[:, b0:b0 + 1]
                        pb = ppool8.tile([128, 512], FP8, tag="pb8")
                    else:
                        for i in range(gn):
                            kt = g0 + i
                            diag = kt == nkt - 1
                            reg = ps[:, 128 * i:128 * (i + 1)]
                            for ci in range(2):
                                nc.tensor.matmul(
                                    reg, kstat(job, ci, kt),
                                    qmov(job, ci, tau),
                                    start=(ci == 0),
                                    stop=(ci == 1 and not diag))
                            if diag:
                                nc.tensor.matmul(reg, ident[:], trineg[:],
                                                 start=False, stop=True)
                        if g0 < P:
                            b0 = 0 if job == "B" else 1 + g0 // 4
                            bias_ap = bias_t[:, b0:b0 + 1]
                        else:
                            bias_ap = bias_t[:, 4:5]
                        pb = (ppool if gi % 2 == 0 else ppool2).tile(
                            [128, 512], BF16, tag="pb")
                    pbs.append((pb, g0, gn, is8))
                    nc.scalar.activation(pb[:, 0:128 * gn], ps[:, 0:128 * gn],
                                         Exp, bias=bias_ap, scale=SCALE)
                for pb, g0, gn, is8 in pbs:
                    if is8:
                        for jj in range(gn // 2):
                            kt = g0 + 2 * jj
                            nc.tensor.matmul(
                                ups[:],
                                pb[:, 256 * jj:256 * (jj + 1)].rearrange(
                                    "p (two f) -> p two f", two=2),
                                (vA8 if job == "A"
                                 else vslabs[job])[kt // 2][:],
                                start=(kt == 0 and job != "A"),
                                stop=False, perf_mode=DR)
                    else:
                        for i in range(gn):
                            kt = g0 + i
                            nc.tensor.matmul(
                                ups[:], pb[:, 128 * i:128 * (i + 1)],
                                vmov(job, kt),
                                start=(kt == 0 and job != "A"),
                                stop=(kt == nkt - 1))

            def run_shared(job, tau):
                ups = ps_small.tile([128, 258], F32, tag="u",
                                    name=f"u{job}{tau}")[:]
                do_qtile(job, tau, ups)
                st = spool.tile([128, 258], BF16, tag="st",
                                name=f"st{job}{tau}")
                nc.vector.tensor_copy(st[:], ups[:])
                st_tiles[job, tau] = st

            # ================= emission schedule =================
            # projections needed for C first, C starts ASAP, then B, then A.
            kproj(0); qproj(0)
            vAp(0); vAp(1)
            kproj(1); qproj(1)
            vAp(2); vAp(3)
            for ch in (6, 7, 8):
                kproj(ch)
            vCpre = [vproj_pair(XA + XB + 256 * j, f"Cp{j}", FP8) for j in range(6)]
            vslabC = vCpre + [vCd_gather(0)]
            vslabs["C"] = vslabC           # vCd[1] appended below

            run_shared("C", 0)
            kproj(2); qproj(2)
            vAp(4); vAp(5)
            run_shared("C", 1)
            kproj(3); qproj(3)
            vAp(6); vAp(7)
            vslabC.append(vCd_gather(1))
            run_shared("C", 2)
            kproj(4); kproj(5)
            vBpre = [vproj_pair(XA + 256 * j, f"Bp{j}", FP8) for j in range(4)]
            run_shared("C", 3)
            vBd = [vBd_gather(j) for j in range(4)]
            vslabs["B"] = vBpre + vBd
            vslabs["A"] = vA

            # ---- B job interleaved with A (A tile t needs st_B[t//2]) ----
            fin = [fpool.tile([128, 2, 256], F32, bufs=1, tag=f"fing{g}",
                              name=f"fing{g}") for g in range(8)]
            out_r = out_d.rearrange("(g t p) c -> g p t c", p=128, t=2)

            def run_A(t):
                ups = ps_small.tile([128, 258], F32, tag="u",
                                    name=f"uA{t}")[:]
                nc.tensor.matmul(ups, pmats[4, t % 4][:],
                                 st_tiles["C", t // 4][:],
                                 start=True, stop=False)
                nc.tensor.matmul(ups, pmats[2, t % 2][:],
                                 st_tiles["B", t // 2][:],
                                 start=False, stop=False)
                do_qtile("A", t, ups)
                g, ti = divmod(t, 2)
                rec = fpool.tile([128, 1], F32, tag="rec")
                nc.vector.reciprocal(rec[:], ups[:, 256:257])
                nc.vector.tensor_scalar_mul(fin[g][:, ti, :],
                                            ups[:, 0:256], rec[:])
                if ti == 1:
                    eng = nc.sync if g % 2 == 0 else nc.scalar
                    eng.dma_start(out_r[g], fin[g][:])

            # deferred fp8 twins of A-region K/Q/V (used by far A tiles)
            for ch in range(4):
                for co in range(2):
                    c0 = CH * ch
                    nc.gpsimd.tensor_copy(kt8a[:, co, c0:c0 + CH],
                                          ktb[:, co, c0:c0 + CH])
                    nc.gpsimd.tensor_copy(qt8a[:, co, c0:c0 + CH],
                                          qtb[:, co, c0:c0 + CH])
            for j in range(8):
                v8 = vpool.tile([128, 2, 258], FP8, tag="vp", name=f"vA8_{j}")
                nc.gpsimd.tensor_copy(v8[:], vA[j][:])
                vA8[j] = v8

            for tau in range(8):
                run_shared("B", tau)
                run_A(2 * tau)
                run_A(2 * tau + 1)

    nc.compile()
    return nc


def _get_program():
    global _PROG
    if _PROG is None:
        _PROG = _build_program()
    return _PROG


def make_in_maps(x, Wq, Wk, Wv):
    """Host-side sharding: gather / transpose / zero-pad / dtype cast only."""
    import ml_dtypes
    bf16 = np.dtype(ml_dtypes.bfloat16)
    x = np.asarray(x, dtype=np.float32)
    w_all = np.concatenate([np.asarray(Wq, np.float32),
                            np.asarray(Wk, np.float32),
                            np.asarray(Wv, np.float32)], axis=1)  # [C, 3C]
    w_t = np.ascontiguousarray(
        w_all.reshape(2, 128, 3 * C).transpose(1, 0, 2).reshape(128, 6 * C)
    ).astype(bf16)

    in_maps = []
    for d in range(NCORES):
        b, q = divmod(d, 4)
        quarter = x[b, 2048 * q:2048 * (q + 1), :]          # [2048, C]

        seg = 0 if q < 2 else 4096
        grp2 = x[b, seg:seg + 4096:2, :]                    # [2048, C]
        r0 = 1024 * (q % 2)
        bpre = grp2[0:1024] if r0 == 1024 else np.zeros((XB, C), np.float32)

        grp4 = x[b, 0:8192:4, :]                            # [2048, C]
        r0c = 512 * q
        cpre = np.concatenate(
            [grp4[0:r0c], np.zeros((XC - r0c, C), np.float32)], axis=0)

        slab = np.concatenate([quarter, bpre, cpre], axis=0)  # [XW, C]
        xt = np.ascontiguousarray(
            slab.T.reshape(2, 128, XW).transpose(1, 0, 2).reshape(128, 2 * XW)
        ).astype(bf16)

        bias = np.zeros((128, 5), np.float32)
        bias[:, 4] = -2.0
        bias[:, 0] = -2.0 if r0 == 1024 else NEG
        for g in range(3):
            bias[:, 1 + g] = -2.0 if g < q else NEG

        in_maps.append({"x": xt, "w": w_t, "bias": bias})
    return in_maps


def kernel(x, Wq, Wk, Wv):
    from concourse.bass_utils import run_bass_kernel_spmd

    nc = _get_program()
    in_maps = make_in_maps(x, Wq, Wk, Wv)
    res = run_bass_kernel_spmd(nc, in_maps, core_ids=list(range(NCORES)))
    out = np.empty((B, N, C), np.float32)
    for d in range(NCORES):
        b, q = divmod(d, 4)
        out[b, 2048 * q:2048 * (q + 1), :] = res.results[d]["out"]
    return out


# revision 7
# speedup vs baseline: 1.3569x; 1.0129x over previous
"""Dilated self-attention TRN2 Bass kernel, v2 (bf16, exact causal shapes).

Problem (hardcoded): B=2, N=8192, C=256, WS=[2048,4096,8192], RS=[1,2,4],
HEAD_IDX=0 -> G=7 groups of s=2048 rows each.

Sharding: core d = (b=d//4, q=d%4) owns output positions [2048q, 2048(q+1))
of batch b.  Jobs per core (all bf16 matmuls, 1 cycle/row at any width):
  C: 4 q-tiles (slots 12..15) of the r=4 group, prefix 12 kt (data-masked)
  B: 8 q-tiles (slots 8..15) of the r=2 half, prefix 8 kt (data-masked)
  A: 16 q-tiles full causal over own quarter
Layout tricks vs v1:
  - One bf16 x slab [128,2,4608] = [A 2048 | Bpre 1024 | Cpre 1536]; B/C
    diag K/Q are strided views into the A region (no re-projection).
  - V tiles for B/C diag are partition-gathered from A's V tiles via
    SBUF->SBUF DMA (stride 2/4) instead of re-projecting.
  - Exact per-q-tile causal prefixes (no 512-block rounding).
  - Scatter-add runs inside the output PSUM accumulation group: placement
    matmuls for B/C land before A's U matmuls; finalize reads PSUM once.
  - Emission order starts the C job while A/B projections still run.
"""

import numpy as np

B, N, C = 2, 8192, 256
S = 2048
NCORES = 8
SCALE = 0.0625
NEG = -1.0e9

XA, XB, XC = 2048, 1024, 1536          # slab section widths
XW = XA + XB + XC                      # 4608

_PROG = None


def _build_program():
    import concourse.mybir as mybir
    import concourse.tile as tile
    from concourse import bacc

    F32 = mybir.dt.float32
    BF16 = mybir.dt.bfloat16
    FP8 = mybir.dt.float8e4
    DR = mybir.MatmulPerfMode.DoubleRow
    Exp = mybir.ActivationFunctionType.Exp

    nc = bacc.Bacc("TRN2", target_bir_lowering=False, debug=False,
                   num_devices=NCORES)

    x_d = nc.dram_tensor("x", [128, 2 * XW], BF16, kind="ExternalInput")
    w_d = nc.dram_tensor("w", [128, 2 * 3 * C], BF16, kind="ExternalInput")
    bias_d = nc.dram_tensor("bias", [128, 5], F32, kind="ExternalInput")
    out_d = nc.dram_tensor("out", [S, C], F32, kind="ExternalOutput")

    with tile.TileContext(nc) as tc:
        with (
            tc.tile_pool(name="const", bufs=1) as cpool,
            tc.tile_pool(name="xsb", bufs=1) as xpool,
            tc.tile_pool(name="kqt", bufs=1) as kqpool,
            tc.tile_pool(name="vext", bufs=32) as vpool,
            tc.tile_pool(name="probs", bufs=6) as ppool,
            tc.tile_pool(name="probs2", bufs=6) as ppool2,
            tc.tile_pool(name="probs8", bufs=6) as ppool8,
            tc.tile_pool(name="stage", bufs=12) as spool,
            tc.tile_pool(name="fin", bufs=4) as fpool,
            tc.tile_pool(name="ps_s", bufs=3, space="PSUM") as ps_scores,
            tc.tile_pool(name="ps_u", bufs=2, space="PSUM") as ps_small,
        ):
            # ---- weights + early x chunks first (PE warmup) ----
            w_sb = cpool.tile([128, 2, 3 * C], BF16, tag="w")
            nc.sync.dma_start(w_sb[:], w_d.rearrange("p (c k) -> p c k", c=2))

            xb = xpool.tile([128, 2, XW], BF16, tag="xb")
            x_r = x_d.rearrange("p (c k) -> p c k", c=2)
            CH = 512
            CHUNK_ORDER = [0, 1, 6, 7, 8, 2, 3, 4, 5]
            for i, ch in enumerate(CHUNK_ORDER):
                eng = nc.sync if i % 2 == 0 else nc.gpsimd
                eng.dma_start(xb[:, :, CH * ch:CH * (ch + 1)],
                              x_r[:, :, CH * ch:CH * (ch + 1)])
            bias_t = cpool.tile([128, 5], F32, tag="bias")
            nc.sync.dma_start(bias_t[:], bias_d[:])

            def wap(name, ci, co):
                i = {"q": 0, "k": 1, "v": 2}[name]
                return w_sb[:, ci, 256 * i + 128 * co:256 * i + 128 * (co + 1)]

            # ---- constants ----
            ones_t = cpool.tile([128, 128], F32, tag="ones")
            nc.gpsimd.memset(ones_t[:], 1.0)
            # identity (stationary) and -1e9 strict-upper (moving): adding
            # ident.T @ trineg to a diag score tile applies the causal mask
            # inside the PSUM accumulation group, PE-side.
            id_f = cpool.tile([128, 128], F32, tag="id_f")
            nc.gpsimd.affine_select(
                out=id_f[:], in_=ones_t[:],
                compare_op=mybir.AluOpType.is_equal,
                fill=0.0, base=0, pattern=[[1, 128]], channel_multiplier=-1)
            ident = cpool.tile([128, 128], BF16, tag="ident")
            nc.vector.tensor_copy(ident[:], id_f[:])
            neg_t = cpool.tile([128, 128], F32, tag="negs")
            nc.gpsimd.memset(neg_t[:], NEG)
            tri_f = cpool.tile([128, 128], F32, tag="tri_f")
            # keep NEG where p > f (mask keys k > query q), 0 elsewhere
            nc.gpsimd.affine_select(
                out=tri_f[:], in_=neg_t[:],
                compare_op=mybir.AluOpType.is_ge,
                fill=0.0, base=-1, pattern=[[-1, 128]], channel_multiplier=1)
            trineg = cpool.tile([128, 128], BF16, tag="trineg")
            nc.vector.tensor_copy(trineg[:], tri_f[:])

            pmats = {}
            for stride, u in [(2, 0), (2, 1), (4, 0), (4, 1), (4, 2), (4, 3)]:
                pf = cpool.tile([128, 128], F32, tag="fscratch",
                                name=f"pmf{stride}_{u}")
                nc.gpsimd.affine_select(
                    out=pf[:], in_=ones_t[:],
                    compare_op=mybir.AluOpType.is_equal,
                    fill=0.0, base=128 * u,
                    pattern=[[1, 128]], channel_multiplier=-stride)
                pm = cpool.tile([128, 128], BF16, tag=f"pm{stride}_{u}")
                nc.vector.tensor_copy(pm[:], pf[:])
                pmats[stride, u] = pm

            # ---- projection helpers (emitted lazily, in job-driven order) --
            ktb = kqpool.tile([128, 2, XA], BF16, tag="ktb")
            qtb = kqpool.tile([128, 2, XA], BF16, tag="qtb")
            kt8 = kqpool.tile([128, 2, XB + XC], FP8, tag="kt8")
            qt8 = kqpool.tile([128, 2, 1536], FP8, tag="qt8")
            kt8a = kqpool.tile([128, 2, XA], FP8, tag="kt8a")
            qt8a = kqpool.tile([128, 2, XA], FP8, tag="qt8a")
            _cp_flip = [0]

            def copy_to(dst, src):
                _cp_flip[0] ^= 1
                if _cp_flip[0]:
                    nc.scalar.copy(dst, src)
                else:
                    nc.vector.tensor_copy(dst, src)

            def kproj(ch):
                c0 = CH * ch
                for co in range(2):
                    ps = ps_small.tile([128, 512], F32, tag="proj", bufs=3)
                    for ci in range(2):
                        nc.tensor.matmul(
                            ps[:], wap("k", ci, co), xb[:, ci, c0:c0 + CH],
                            start=(ci == 0), stop=(ci == 1))
                    if c0 >= XA:
                        nc.vector.tensor_copy(
                            kt8[:, co, c0 - XA:c0 - XA + CH], ps[:])
                    else:
                        copy_to(ktb[:, co, c0:c0 + CH], ps[:])

            def qproj(ch):
                c0 = CH * ch
                for co in range(2):
                    ps = ps_small.tile([128, 512], F32, tag="proj", bufs=3)
                    for ci in range(2):
                        nc.tensor.matmul(
                            ps[:], wap("q", ci, co), xb[:, ci, c0:c0 + CH],
                            start=(ci == 0), stop=(ci == 1))
                    copy_to(qtb[:, co, c0:c0 + CH], ps[:])
                    nc.gpsimd.tensor_copy(
                        qt8[:, co, c0 // 2:c0 // 2 + 256],
                        qtb[:, co, c0:c0 + CH:2])
                    nc.gpsimd.tensor_copy(
                        qt8[:, co, 1024 + c0 // 4:1024 + c0 // 4 + 128],
                        qtb[:, co, c0:c0 + CH:4])

            def vproj_pair(src_c0, name, dt=BF16):
                """project x cols [src_c0, src_c0+256) -> one [128,2,258]"""
                ps = ps_small.tile([128, 512], F32, tag="proj", bufs=3,
                                   name=f"psv{name}")
                for half in range(2):
                    for ci in range(2):
                        nc.tensor.matmul(
                            ps[:, 256 * half:256 * (half + 1)],
                            xb[:, ci,
                               src_c0 + 128 * half:src_c0 + 128 * (half + 1)],
                            w_sb[:, ci, 512:768],
                            start=(ci == 0), stop=(ci == 1))
                v = vpool.tile([128, 2, 258], dt, tag="vp", name=f"vp{name}")
                if dt is BF16:
                    copy_to(v[:, :, 0:256],
                            ps.rearrange("p (h c) -> p h c", h=2))
                else:
                    nc.vector.tensor_copy(v[:, :, 0:256],
                                          ps.rearrange("p (h c) -> p h c", h=2))
                nc.vector.memset(v[:, :, 256:257], 1.0)
                nc.vector.memset(v[:, :, 257:258], 0.0)
                return v

            vA = [None] * 8
            vA8 = [None] * 8

            def vAp(j):
                if vA[j] is None:
                    vA[j] = vproj_pair(256 * j, f"A{j}")
                return vA[j]

            def vBd_gather(j):
                """B diag pair j rows = A rows [512j, 512j+512) step 2."""
                v = vpool.tile([128, 2, 258], BF16, tag="vp", name=f"vBd{j}")
                for half in range(2):
                    for s in range(2):
                        src = vAp(2 * j + half)
                        nc.sync.dma_start(
                            v[64 * s:64 * (s + 1), half, :],
                            src[0:128:2, s, :])
                return v

            def vCd_gather(j):
                """C diag pair j rows = A rows [1024j, 1024j+1024) step 4."""
                v = vpool.tile([128, 2, 258], BF16, tag="vp", name=f"vCd{j}")
                for half in range(2):
                    base = 512 * (2 * j + half)
                    for s in range(4):
                        src = vAp((base + 128 * s) // 256)
                        sh = ((base + 128 * s) // 128) % 2
                        nc.sync.dma_start(
                            v[32 * s:32 * (s + 1), half, :],
                            src[0:128:4, sh, :])
                return v

            jobs = {
                "C": dict(nq=4, P=12, stride=4, kpre=XA + XB, dstep=4),
                "B": dict(nq=8, P=8, stride=2, kpre=XA, dstep=2),
                "A": dict(nq=16, P=0, stride=1, kpre=None, dstep=1),
            }
            vslabs = {}

            def kstat(job, ci, kt):
                j = jobs[job]
                if kt < j["P"]:
                    c0 = j["kpre"] + 128 * kt
                    return ktb[:, ci, c0:c0 + 128]
                d = kt - j["P"]
                st = j["dstep"]
                return ktb[:, ci, 128 * st * d:128 * st * (d + 1):st]

            def qmov(job, ci, tau):
                st = jobs[job]["dstep"]
                return qtb[:, ci, 128 * st * tau:128 * st * (tau + 1):st]

            def kstat8(job, kt):
                if job == "A":
                    return kt8a[:, :, 128 * kt:128 * (kt + 1)]
                off = 0 if job == "B" else XB
                return kt8[:, :, off + 128 * kt:off + 128 * (kt + 1)]

            def qmov8(job, tau):
                if job == "A":
                    return qt8a[:, :, 128 * tau:128 * (tau + 1)]
                off = 0 if job == "B" else 1024
                return qt8[:, :, off + 128 * tau:off + 128 * (tau + 1)]

            def vmov(job, kt):
                j = vslabs[job][kt // 2]
                return j[:, kt % 2, :]

            st_tiles = {}

            def do_qtile(job, tau, ups):
                """scores+exp+U for q-tile tau of job; U accumulated into
                psum tile `ups` (A: group already started by placements)."""
                j = jobs[job]
                P = j["P"]
                nkt = P + tau + 1
                pbs = []            # (pb_tile, kt0, nkt_in_group, is8)
                for gi, g0 in enumerate(range(0, nkt, 4)):
                    gn = min(4, nkt - g0)
                    # groups fully inside the prefix use the fp8 DoubleRow
                    # path (long-range diffuse attention tolerates fp8)
                    if job == "A":
                        is8 = g0 + gn - 1 <= tau - 5
                    else:
                        is8 = g0 + gn <= P
                    ps = ps_scores.tile([128, 512], F32, tag="scores")
                    if is8:
                        for i in range(gn):
                            kt = g0 + i
                            nc.tensor.matmul(
                                ps[:, 128 * i:128 * (i + 1)],
                                kstat8(job, kt), qmov8(job, tau),
                                start=True, stop=True, perf_mode=DR)
                        if job == "A":
                            bias_ap = bias_t[:, 4:5]
                        else:
                            b0 = 0 if job == "B" else 1 + g0 // 4
                            bias_ap = bias_t[:, b0:b0 + 1]
                        pb = ppool8.tile([128, 512], FP8, tag="pb8")
                    else:
                        for i in range(gn):
                            kt = g0 + i
                            diag = kt == nkt - 1
                            reg = ps[:, 128 * i:128 * (i + 1)]
                            for ci in range(2):
                                nc.tensor.matmul(
                                    reg, kstat(job, ci, kt),
                                    qmov(job, ci, tau),
                                    start=(ci == 0),
                                    stop=(ci == 1 and not diag))
                            if diag:
                                nc.tensor.matmul(reg, ident[:], trineg[:],
                                                 start=False, stop=True)
                        if g0 < P:
                            b0 = 0 if job == "B" else 1 + g0 // 4
                            bias_ap = bias_t[:, b0:b0 + 1]
                        else:
                            bias_ap = bias_t[:, 4:5]
                        pb = (ppool if gi % 2 == 0 else ppool2).tile(
                            [128, 512], BF16, tag="pb")
                    pbs.append((pb, g0, gn, is8))
                    nc.scalar.activation(pb[:, 0:128 * gn], ps[:, 0:128 * gn],
                                         Exp, bias=bias_ap, scale=SCALE)
                for pb, g0, gn, is8 in pbs:
                    if is8:
                        for jj in range(gn // 2):
                            kt = g0 + 2 * jj
                            nc.tensor.matmul(
                                ups[:],
                                pb[:, 256 * jj:256 * (jj + 1)].rearrange(
                                    "p (two f) -> p two f", two=2),
                                (vA8 if job == "A"
                                 else vslabs[job])[kt // 2][:],
                                start=(kt == 0 and job != "A"),
                                stop=False, perf_mode=DR)
                    else:
                        for i in range(gn):
                            kt = g0 + i
                            nc.tensor.matmul(
                                ups[:], pb[:, 128 * i:128 * (i + 1)],
                                vmov(job, kt),
                                start=(kt == 0 and job != "A"),
                                stop=(kt == nkt - 1))

            def run_shared(job, tau):
                ups = ps_small.tile([128, 258], F32, tag="u",
                                    name=f"u{job}{tau}")[:]
                do_qtile(job, tau, ups)
                st = spool.tile([128, 258], BF16, tag="st",
                                name=f"st{job}{tau}")
                nc.vector.tensor_copy(st[:], ups[:])
                st_tiles[job, tau] = st

            # ================= emission schedule =================
            # projections needed for C first, C starts ASAP, then B, then A.
            kproj(0); qproj(0)
            vAp(0); vAp(1)
            kproj(1); qproj(1)
            vAp(2); vAp(3)
            for ch in (6, 7, 8):
                kproj(ch)
            vCpre = [vproj_pair(XA + XB + 256 * j, f"Cp{j}", FP8) for j in range(6)]
            vslabC = vCpre + [vCd_gather(0)]
            vslabs["C"] = vslabC           # vCd[1] appended below

            run_shared("C", 0)
            kproj(2); qproj(2)
            vAp(4); vAp(5)
            run_shared("C", 1)
            kproj(3); qproj(3)
            vAp(6); vAp(7)
            vslabC.append(vCd_gather(1))
            run_shared("C", 2)
            kproj(4); kproj(5)
            vBpre = [vproj_pair(XA + 256 * j, f"Bp{j}", FP8) for j in range(4)]
            run_shared("C", 3)
            vBd = [vBd_gather(j) for j in range(4)]
            vslabs["B"] = vBpre + vBd
            vslabs["A"] = vA

            # ---- B job interleaved with A (A tile t needs st_B[t//2]) ----
            fin = [fpool.tile([128, 2, 256], F32, bufs=1, tag=f"fing{g}",
                              name=f"fing{g}") for g in range(8)]
            out_r = out_d.rearrange("(g t p) c -> g p t c", p=128, t=2)

            def run_A(t):
                ups = ps_small.tile([128, 258], F32, tag="u",
                                    name=f"uA{t}")[:]
                nc.tensor.matmul(ups, pmats[4, t % 4][:],
                                 st_tiles["C", t // 4][:],
                                 start=True, stop=False)
                nc.tensor.matmul(ups, pmats[2, t % 2][:],
                                 st_tiles["B", t // 2][:],
                                 start=False, stop=False)
                do_qtile("A", t, ups)
                g, ti = divmod(t, 2)
                rec = fpool.tile([128, 1], F32, tag="rec")
                nc.vector.reciprocal(rec[:], ups[:, 256:257])
                nc.vector.tensor_scalar_mul(fin[g][:, ti, :],
                                            ups[:, 0:256], rec[:])
                if ti == 1:
                    eng = nc.sync if g % 2 == 0 else nc.gpsimd
                    eng.dma_start(out_r[g], fin[g][:])

            # deferred fp8 twins of A-region K/Q/V (used by far A tiles)
            for ch in range(4):
                for co in range(2):
                    c0 = CH * ch
                    nc.gpsimd.tensor_copy(kt8a[:, co, c0:c0 + CH],
                                          ktb[:, co, c0:c0 + CH])
                    nc.gpsimd.tensor_copy(qt8a[:, co, c0:c0 + CH],
                                          qtb[:, co, c0:c0 + CH])
            for j in range(8):
                v8 = vpool.tile([128, 2, 258], FP8, tag="vp", name=f"vA8_{j}")
                nc.gpsimd.tensor_copy(v8[:], vA[j][:])
                vA8[j] = v8

            for tau in range(8):
                run_shared("B", tau)
                run_A(2 * tau)
                run_A(2 * tau + 1)

    nc.compile()
    return nc


def _get_program():
    global _PROG
    if _PROG is None:
        _PROG = _build_program()
    return _PROG


def make_in_maps(x, Wq, Wk, Wv):
    """Host-side sharding: gather / transpose / zero-pad / dtype cast only."""
    import ml_dtypes
    bf16 = np.dtype(ml_dtypes.bfloat16)
    x = np.asarray(x, dtype=np.float32)
    w_all = np.concatenate([np.asarray(Wq, np.float32),
                            np.asarray(Wk, np.float32),
                            np.asarray(Wv, np.float32)], axis=1)  # [C, 3C]
    w_t = np.ascontiguousarray(
        w_all.reshape(2, 128, 3 * C).transpose(1, 0, 2).reshape(128, 6 * C)
    ).astype(bf16)

    in_maps = []
    for d in range(NCORES):
        b, q = divmod(d, 4)
        quarter = x[b, 2048 * q:2048 * (q + 1), :]          # [2048, C]

        seg = 0 if q < 2 else 4096
        grp2 = x[b, seg:seg + 4096:2, :]                    # [2048, C]
        r0 = 1024 * (q % 2)
        bpre = grp2[0:1024] if r0 == 1024 else np.zeros((XB, C), np.float32)

        grp4 = x[b, 0:8192:4, :]                            # [2048, C]
        r0c = 512 * q
        cpre = np.concatenate(
            [grp4[0:r0c], np.zeros((XC - r0c, C), np.float32)], axis=0)

        slab = np.concatenate([quarter, bpre, cpre], axis=0)  # [XW, C]
        xt = np.ascontiguousarray(
            slab.T.reshape(2, 128, XW).transpose(1, 0, 2).reshape(128, 2 * XW)
        ).astype(bf16)

        bias = np.zeros((128, 5), np.float32)
        bias[:, 4] = -2.0
        bias[:, 0] = -2.0 if r0 == 1024 else NEG
        for g in range(3):
            bias[:, 1 + g] = -2.0 if g < q else NEG

        in_maps.append({"x": xt, "w": w_t, "bias": bias})
    return in_maps


def kernel(x, Wq, Wk, Wv):
    from concourse.bass_utils import run_bass_kernel_spmd

    nc = _get_program()
    in_maps = make_in_maps(x, Wq, Wk, Wv)
    res = run_bass_kernel_spmd(nc, in_maps, core_ids=list(range(NCORES)))
    out = np.empty((B, N, C), np.float32)
    for d in range(NCORES):
        b, q = divmod(d, 4)
        out[b, 2048 * q:2048 * (q + 1), :] = res.results[d]["out"]
    return out


# revision 9
# speedup vs baseline: 1.3827x; 1.0190x over previous
"""Dilated self-attention TRN2 Bass kernel, v2 (bf16, exact causal shapes).

Problem (hardcoded): B=2, N=8192, C=256, WS=[2048,4096,8192], RS=[1,2,4],
HEAD_IDX=0 -> G=7 groups of s=2048 rows each.

Sharding: core d = (b=d//4, q=d%4) owns output positions [2048q, 2048(q+1))
of batch b.  Jobs per core (all bf16 matmuls, 1 cycle/row at any width):
  C: 4 q-tiles (slots 12..15) of the r=4 group, prefix 12 kt (data-masked)
  B: 8 q-tiles (slots 8..15) of the r=2 half, prefix 8 kt (data-masked)
  A: 16 q-tiles full causal over own quarter
Layout tricks vs v1:
  - One bf16 x slab [128,2,4608] = [A 2048 | Bpre 1024 | Cpre 1536]; B/C
    diag K/Q are strided views into the A region (no re-projection).
  - V tiles for B/C diag are partition-gathered from A's V tiles via
    SBUF->SBUF DMA (stride 2/4) instead of re-projecting.
  - Exact per-q-tile causal prefixes (no 512-block rounding).
  - Scatter-add runs inside the output PSUM accumulation group: placement
    matmuls for B/C land before A's U matmuls; finalize reads PSUM once.
  - Emission order starts the C job while A/B projections still run.
"""

import numpy as np

B, N, C = 2, 8192, 256
S = 2048
NCORES = 8
SCALE = 0.0625
NEG = -1.0e9

XA, XB, XC = 2048, 1024, 1536          # slab section widths
XW = XA + XB + XC                      # 4608

_PROG = None


def _build_program():
    import concourse.mybir as mybir
    import concourse.tile as tile
    from concourse import bacc

    F32 = mybir.dt.float32
    BF16 = mybir.dt.bfloat16
    FP8 = mybir.dt.float8e4
    DR = mybir.MatmulPerfMode.DoubleRow
    Exp = mybir.ActivationFunctionType.Exp

    nc = bacc.Bacc("TRN2", target_bir_lowering=False, debug=False,
                   num_devices=NCORES)

    x_d = nc.dram_tensor("x", [128, 2 * XW], BF16, kind="ExternalInput")
    w_d = nc.dram_tensor("w", [128, 2 * 3 * C], BF16, kind="ExternalInput")
    bias_d = nc.dram_tensor("bias", [128, 5], F32, kind="ExternalInput")
    out_d = nc.dram_tensor("out", [S, C], F32, kind="ExternalOutput")

    with tile.TileContext(nc) as tc:
        with (
            tc.tile_pool(name="const", bufs=1) as cpool,
            tc.tile_pool(name="xsb", bufs=1) as xpool,
            tc.tile_pool(name="kqt", bufs=1) as kqpool,
            tc.tile_pool(name="vext", bufs=32) as vpool,
            tc.tile_pool(name="probs", bufs=6) as ppool,
            tc.tile_pool(name="probs2", bufs=6) as ppool2,
            tc.tile_pool(name="probs8", bufs=6) as ppool8,
            tc.tile_pool(name="stage", bufs=12) as spool,
            tc.tile_pool(name="fin", bufs=4) as fpool,
            tc.tile_pool(name="ps_s", bufs=3, space="PSUM") as ps_scores,
            tc.tile_pool(name="ps_u", bufs=2, space="PSUM") as ps_small,
        ):
            # ---- weights + early x chunks first (PE warmup) ----
            w_sb = cpool.tile([128, 2, 3 * C], BF16, tag="w")
            nc.sync.dma_start(w_sb[:], w_d.rearrange("p (c k) -> p c k", c=2))

            xb = xpool.tile([128, 2, XW], BF16, tag="xb")
            x_r = x_d.rearrange("p (c k) -> p c k", c=2)
            CH = 512
            CHUNK_ORDER = [0, 1, 6, 7, 8, 2, 3, 4, 5]
            for i, ch in enumerate(CHUNK_ORDER):
                eng = nc.sync if i % 2 == 0 else nc.gpsimd
                eng.dma_start(xb[:, :, CH * ch:CH * (ch + 1)],
                              x_r[:, :, CH * ch:CH * (ch + 1)])
            bias_t = cpool.tile([128, 5], F32, tag="bias")
            nc.sync.dma_start(bias_t[:], bias_d[:])

            def wap(name, ci, co):
                i = {"q": 0, "k": 1, "v": 2}[name]
                return w_sb[:, ci, 256 * i + 128 * co:256 * i + 128 * (co + 1)]

            # ---- constants ----
            ones_t = cpool.tile([128, 128], F32, tag="ones")
            nc.gpsimd.memset(ones_t[:], 1.0)
            # identity (stationary) and -1e9 strict-upper (moving): adding
            # ident.T @ trineg to a diag score tile applies the causal mask
            # inside the PSUM accumulation group, PE-side.
            id_f = cpool.tile([128, 128], F32, tag="id_f")
            nc.gpsimd.affine_select(
                out=id_f[:], in_=ones_t[:],
                compare_op=mybir.AluOpType.is_equal,
                fill=0.0, base=0, pattern=[[1, 128]], channel_multiplier=-1)
            ident = cpool.tile([128, 128], BF16, tag="ident")
            nc.vector.tensor_copy(ident[:], id_f[:])
            neg_t = cpool.tile([128, 128], F32, tag="negs")
            nc.gpsimd.memset(neg_t[:], NEG)
            tri_f = cpool.tile([128, 128], F32, tag="tri_f")
            # keep NEG where p > f (mask keys k > query q), 0 elsewhere
            nc.gpsimd.affine_select(
                out=tri_f[:], in_=neg_t[:],
                compare_op=mybir.AluOpType.is_ge,
                fill=0.0, base=-1, pattern=[[-1, 128]], channel_multiplier=1)
            trineg = cpool.tile([128, 128], BF16, tag="trineg")
            nc.vector.tensor_copy(trineg[:], tri_f[:])

            pmats = {}
            for stride, u in [(2, 0), (2, 1), (4, 0), (4, 1), (4, 2), (4, 3)]:
                pf = cpool.tile([128, 128], F32, tag="fscratch",
                                name=f"pmf{stride}_{u}")
                nc.gpsimd.affine_select(
                    out=pf[:], in_=ones_t[:],
                    compare_op=mybir.AluOpType.is_equal,
                    fill=0.0, base=128 * u,
                    pattern=[[1, 128]], channel_multiplier=-stride)
                pm = cpool.tile([128, 128], BF16, tag=f"pm{stride}_{u}")
                nc.vector.tensor_copy(pm[:], pf[:])
                pmats[stride, u] = pm

            # ---- projection helpers (emitted lazily, in job-driven order) --
            ktb = kqpool.tile([128, 2, XA], BF16, tag="ktb")
            qtb = kqpool.tile([128, 2, XA], BF16, tag="qtb")
            kt8 = kqpool.tile([128, 2, XB + XC], FP8, tag="kt8")
            qt8 = kqpool.tile([128, 2, 1536], FP8, tag="qt8")
            kt8a = kqpool.tile([128, 2, XA], FP8, tag="kt8a")
            qt8a = kqpool.tile([128, 2, XA], FP8, tag="qt8a")
            _cp_flip = [0]

            def copy_to(dst, src):
                _cp_flip[0] ^= 1
                if _cp_flip[0]:
                    nc.scalar.copy(dst, src)
                else:
                    nc.vector.tensor_copy(dst, src)

            def kproj(ch):
                c0 = CH * ch
                for co in range(2):
                    ps = ps_small.tile([128, 512], F32, tag="proj", bufs=3)
                    for ci in range(2):
                        nc.tensor.matmul(
                            ps[:], wap("k", ci, co), xb[:, ci, c0:c0 + CH],
                            start=(ci == 0), stop=(ci == 1))
                    if c0 >= XA:
                        copy_to(kt8[:, co, c0 - XA:c0 - XA + CH], ps[:])
                    else:
                        copy_to(ktb[:, co, c0:c0 + CH], ps[:])

            def qproj(ch):
                c0 = CH * ch
                for co in range(2):
                    ps = ps_small.tile([128, 512], F32, tag="proj", bufs=3)
                    for ci in range(2):
                        nc.tensor.matmul(
                            ps[:], wap("q", ci, co), xb[:, ci, c0:c0 + CH],
                            start=(ci == 0), stop=(ci == 1))
                    copy_to(qtb[:, co, c0:c0 + CH], ps[:])
                    nc.gpsimd.tensor_copy(
                        qt8[:, co, c0 // 2:c0 // 2 + 256],
                        qtb[:, co, c0:c0 + CH:2])
                    nc.gpsimd.tensor_copy(
                        qt8[:, co, 1024 + c0 // 4:1024 + c0 // 4 + 128],
                        qtb[:, co, c0:c0 + CH:4])

            def vproj_pair(src_c0, name, dt=BF16):
                """project x cols [src_c0, src_c0+256) -> one [128,2,258]"""
                ps = ps_small.tile([128, 512], F32, tag="proj", bufs=3,
                                   name=f"psv{name}")
                for half in range(2):
                    for ci in range(2):
                        nc.tensor.matmul(
                            ps[:, 256 * half:256 * (half + 1)],
                            xb[:, ci,
                               src_c0 + 128 * half:src_c0 + 128 * (half + 1)],
                            w_sb[:, ci, 512:768],
                            start=(ci == 0), stop=(ci == 1))
                v = vpool.tile([128, 2, 258], dt, tag="vp", name=f"vp{name}")
                if dt is BF16:
                    copy_to(v[:, :, 0:256],
                            ps.rearrange("p (h c) -> p h c", h=2))
                else:
                    copy_to(v[:, :, 0:256],
                            ps.rearrange("p (h c) -> p h c", h=2))
                nc.vector.memset(v[:, :, 256:257], 1.0)
                nc.vector.memset(v[:, :, 257:258], 0.0)
                return v

            vA = [None] * 8
            vA8 = [None] * 8

            def vAp(j):
                if vA[j] is None:
                    vA[j] = vproj_pair(256 * j, f"A{j}")
                return vA[j]

            def vBd_gather(j):
                """B diag pair j rows = A rows [512j, 512j+512) step 2."""
                v = vpool.tile([128, 2, 258], BF16, tag="vp", name=f"vBd{j}")
                for half in range(2):
                    for s in range(2):
                        src = vAp(2 * j + half)
                        nc.sync.dma_start(
                            v[64 * s:64 * (s + 1), half, :],
                            src[0:128:2, s, :])
                return v

            def vCd_gather(j):
                """C diag pair j rows = A rows [1024j, 1024j+1024) step 4."""
                v = vpool.tile([128, 2, 258], BF16, tag="vp", name=f"vCd{j}")
                for half in range(2):
                    base = 512 * (2 * j + half)
                    for s in range(4):
                        src = vAp((base + 128 * s) // 256)
                        sh = ((base + 128 * s) // 128) % 2
                        nc.sync.dma_start(
                            v[32 * s:32 * (s + 1), half, :],
                            src[0:128:4, sh, :])
                return v

            jobs = {
                "C": dict(nq=4, P=12, stride=4, kpre=XA + XB, dstep=4),
                "B": dict(nq=8, P=8, stride=2, kpre=XA, dstep=2),
                "A": dict(nq=16, P=0, stride=1, kpre=None, dstep=1),
            }
            vslabs = {}

            def kstat(job, ci, kt):
                j = jobs[job]
                if kt < j["P"]:
                    c0 = j["kpre"] + 128 * kt
                    return ktb[:, ci, c0:c0 + 128]
                d = kt - j["P"]
                st = j["dstep"]
                return ktb[:, ci, 128 * st * d:128 * st * (d + 1):st]

            def qmov(job, ci, tau):
                st = jobs[job]["dstep"]
                return qtb[:, ci, 128 * st * tau:128 * st * (tau + 1):st]

            def kstat8(job, kt):
                if job == "A":
                    return kt8a[:, :, 128 * kt:128 * (kt + 1)]
                off = 0 if job == "B" else XB
                return kt8[:, :, off + 128 * kt:off + 128 * (kt + 1)]

            def qmov8(job, tau):
                if job == "A":
                    return qt8a[:, :, 128 * tau:128 * (tau + 1)]
                off = 0 if job == "B" else 1024
                return qt8[:, :, off + 128 * tau:off + 128 * (tau + 1)]

            def vmov(job, kt):
                j = vslabs[job][kt // 2]
                return j[:, kt % 2, :]

            st_tiles = {}

            def do_qtile(job, tau, ups):
                """scores+exp+U for q-tile tau of job; U accumulated into
                psum tile `ups` (A: group already started by placements)."""
                j = jobs[job]
                P = j["P"]
                nkt = P + tau + 1
                pbs = []            # (pb_tile, kt0, nkt_in_group, is8)
                for gi, g0 in enumerate(range(0, nkt, 4)):
                    gn = min(4, nkt - g0)
                    # groups fully inside the prefix use the fp8 DoubleRow
                    # path (long-range diffuse attention tolerates fp8)
                    if job == "A":
                        is8 = g0 + gn - 1 <= tau - 5
                    else:
                        is8 = g0 + gn <= P
                    ps = ps_scores.tile([128, 512], F32, tag="scores")
                    if is8:
                        for i in range(gn):
                            kt = g0 + i
                            nc.tensor.matmul(
                                ps[:, 128 * i:128 * (i + 1)],
                                kstat8(job, kt), qmov8(job, tau),
                                start=True, stop=True, perf_mode=DR)
                        if job == "A":
                            bias_ap = bias_t[:, 4:5]
                        else:
                            b0 = 0 if job == "B" else 1 + g0 // 4
                            bias_ap = bias_t[:, b0:b0 + 1]
                        pb = ppool8.tile([128, 512], FP8, tag="pb8")
                    else:
                        for i in range(gn):
                            kt = g0 + i
                            diag = kt == nkt - 1
                            reg = ps[:, 128 * i:128 * (i + 1)]
                            for ci in range(2):
                                nc.tensor.matmul(
                                    reg, kstat(job, ci, kt),
                                    qmov(job, ci, tau),
                                    start=(ci == 0),
                                    stop=(ci == 1 and not diag))
                            if diag:
                                nc.tensor.matmul(reg, ident[:], trineg[:],
                                                 start=False, stop=True)
                        if g0 < P:
                            b0 = 0 if job == "B" else 1 + g0 // 4
                            bias_ap = bias_t[:, b0:b0 + 1]
                        else:
                            bias_ap = bias_t[:, 4:5]
                        pb = (ppool if gi % 2 == 0 else ppool2).tile(
                            [128, 512], BF16, tag="pb")
                    pbs.append((pb, g0, gn, is8))
                    nc.scalar.activation(pb[:, 0:128 * gn], ps[:, 0:128 * gn],
                                         Exp, bias=bias_ap, scale=SCALE)
                for pb, g0, gn, is8 in pbs:
                    if is8:
                        for jj in range(gn // 2):
                            kt = g0 + 2 * jj
                            nc.tensor.matmul(
                                ups[:],
                                pb[:, 256 * jj:256 * (jj + 1)].rearrange(
                                    "p (two f) -> p two f", two=2),
                                (vA8 if job == "A"
                                 else vslabs[job])[kt // 2][:],
                                start=(kt == 0 and job != "A"),
                                stop=False, perf_mode=DR)
                    else:
                        for i in range(gn):
                            kt = g0 + i
                            nc.tensor.matmul(
                                ups[:], pb[:, 128 * i:128 * (i + 1)],
                                vmov(job, kt),
                                start=(kt == 0 and job != "A"),
                                stop=(kt == nkt - 1))

            def run_shared(job, tau):
                ups = ps_small.tile([128, 258], F32, tag="u",
                                    name=f"u{job}{tau}")[:]
                do_qtile(job, tau, ups)
                st = spool.tile([128, 258], BF16, tag="st",
                                name=f"st{job}{tau}")
                nc.vector.tensor_copy(st[:], ups[:])
                st_tiles[job, tau] = st

            # ================= emission schedule =================
            # projections needed for C first, C starts ASAP, then B, then A.
            kproj(0); qproj(0)
            vAp(0); vAp(1)
            kproj(1); qproj(1)
            vAp(2); vAp(3)
            for ch in (6, 7, 8):
                kproj(ch)
            vCpre = [vproj_pair(XA + XB + 256 * j, f"Cp{j}", FP8) for j in range(6)]
            vslabC = vCpre + [vCd_gather(0)]
            vslabs["C"] = vslabC           # vCd[1] appended below

            run_shared("C", 0)
            kproj(2); qproj(2)
            vAp(4); vAp(5)
            run_shared("C", 1)
            kproj(3); qproj(3)
            vAp(6); vAp(7)
            vslabC.append(vCd_gather(1))
            run_shared("C", 2)
            kproj(4); kproj(5)
            vBpre = [vproj_pair(XA + 256 * j, f"Bp{j}", FP8) for j in range(4)]
            run_shared("C", 3)
            vBd = [vBd_gather(j) for j in range(4)]
            vslabs["B"] = vBpre + vBd
            vslabs["A"] = vA

            # ---- B job interleaved with A (A tile t needs st_B[t//2]) ----
            fin = [fpool.tile([128, 2, 256], F32, bufs=1, tag=f"fing{g}",
                              name=f"fing{g}") for g in range(8)]
            out_r = out_d.rearrange("(g t p) c -> g p t c", p=128, t=2)

            def run_A(t):
                ups = ps_small.tile([128, 258], F32, tag="u",
                                    name=f"uA{t}")[:]
                nc.tensor.matmul(ups, pmats[4, t % 4][:],
                                 st_tiles["C", t // 4][:],
                                 start=True, stop=False)
                nc.tensor.matmul(ups, pmats[2, t % 2][:],
                                 st_tiles["B", t // 2][:],
                                 start=False, stop=False)
                do_qtile("A", t, ups)
                g, ti = divmod(t, 2)
                rec = fpool.tile([128, 1], F32, tag="rec")
                nc.vector.reciprocal(rec[:], ups[:, 256:257])
                nc.vector.tensor_scalar_mul(fin[g][:, ti, :],
                                            ups[:, 0:256], rec[:])
                if ti == 1:
                    eng = nc.sync if g % 2 == 0 else nc.gpsimd
                    eng.dma_start(out_r[g], fin[g][:])

            # deferred fp8 twins of A-region K/Q/V (used by far A tiles,
            # first at A t=8 -> spread emission across early B iterations)
            def emit_twins(part):
                for ch in (part, part + 2):
                    for co in range(2):
                        c0 = CH * ch
                        nc.gpsimd.tensor_copy(kt8a[:, co, c0:c0 + CH],
                                              ktb[:, co, c0:c0 + CH])
                        nc.gpsimd.tensor_copy(qt8a[:, co, c0:c0 + CH],
                                              qtb[:, co, c0:c0 + CH])
                for j in (4 * part, 4 * part + 1, 4 * part + 2, 4 * part + 3):
                    v8 = vpool.tile([128, 2, 258], FP8, tag="vp",
                                    name=f"vA8_{j}")
                    nc.gpsimd.tensor_copy(v8[:], vA[j][:])
                    vA8[j] = v8

            for tau in range(8):
                run_shared("B", tau)
                if tau < 2:
                    emit_twins(tau)
                run_A(2 * tau)
                run_A(2 * tau + 1)

    nc.compile()
    return nc


def _get_program():
    global _PROG
    if _PROG is None:
        _PROG = _build_program()
    return _PROG


def make_in_maps(x, Wq, Wk, Wv):
    """Host-side sharding: gather / transpose / zero-pad / dtype cast only."""
    import ml_dtypes
    bf16 = np.dtype(ml_dtypes.bfloat16)
    x = np.asarray(x, dtype=np.float32)
    w_all = np.concatenate([np.asarray(Wq, np.float32),
                            np.asarray(Wk, np.float32),
                            np.asarray(Wv, np.float32)], axis=1)  # [C, 3C]
    w_t = np.ascontiguousarray(
        w_all.reshape(2, 128, 3 * C).transpose(1, 0, 2).reshape(128, 6 * C)
    ).astype(bf16)

    in_maps = []
    for d in range(NCORES):
        b, q = divmod(d, 4)
        quarter = x[b, 2048 * q:2048 * (q + 1), :]          # [2048, C]

        seg = 0 if q < 2 else 4096
        grp2 = x[b, seg:seg + 4096:2, :]                    # [2048, C]
        r0 = 1024 * (q % 2)
        bpre = grp2[0:1024] if r0 == 1024 else np.zeros((XB, C), np.float32)

        grp4 = x[b, 0:8192:4, :]                            # [2048, C]
        r0c = 512 * q
        cpre = np.concatenate(
            [grp4[0:r0c], np.zeros((XC - r0c, C), np.float32)], axis=0)

        slab = np.concatenate([quarter, bpre, cpre], axis=0)  # [XW, C]
        xt = np.ascontiguousarray(
            slab.T.reshape(2, 128, XW).transpose(1, 0, 2).reshape(128, 2 * XW)
        ).astype(bf16)

        bias = np.zeros((128, 5), np.float32)
        bias[:, 4] = -2.0
        bias[:, 0] = -2.0 if r0 == 1024 else NEG
        for g in range(3):
            bias[:, 1 + g] = -2.0 if g < q else NEG

        in_maps.append({"x": xt, "w": w_t, "bias": bias})
    return in_maps


def kernel(x, Wq, Wk, Wv):
    from concourse.bass_utils import run_bass_kernel_spmd

    nc = _get_program()
    in_maps = make_in_maps(x, Wq, Wk, Wv)
    res = run_bass_kernel_spmd(nc, in_maps, core_ids=list(range(NCORES)))
    out = np.empty((B, N, C), np.float32)
    for d in range(NCORES):
        b, q = divmod(d, 4)
        out[b, 2048 * q:2048 * (q + 1), :] = res.results[d]["out"]
    return out


# revision 10
# speedup vs baseline: 1.3865x; 1.0027x over previous
"""Dilated self-attention TRN2 Bass kernel, v2 (bf16, exact causal shapes).

Problem (hardcoded): B=2, N=8192, C=256, WS=[2048,4096,8192], RS=[1,2,4],
HEAD_IDX=0 -> G=7 groups of s=2048 rows each.

Sharding: core d = (b=d//4, q=d%4) owns output positions [2048q, 2048(q+1))
of batch b.  Jobs per core (all bf16 matmuls, 1 cycle/row at any width):
  C: 4 q-tiles (slots 12..15) of the r=4 group, prefix 12 kt (data-masked)
  B: 8 q-tiles (slots 8..15) of the r=2 half, prefix 8 kt (data-masked)
  A: 16 q-tiles full causal over own quarter
Layout tricks vs v1:
  - One bf16 x slab [128,2,4608] = [A 2048 | Bpre 1024 | Cpre 1536]; B/C
    diag K/Q are strided views into the A region (no re-projection).
  - V tiles for B/C diag are partition-gathered from A's V tiles via
    SBUF->SBUF DMA (stride 2/4) instead of re-projecting.
  - Exact per-q-tile causal prefixes (no 512-block rounding).
  - Scatter-add runs inside the output PSUM accumulation group: placement
    matmuls for B/C land before A's U matmuls; finalize reads PSUM once.
  - Emission order starts the C job while A/B projections still run.
"""

import numpy as np

B, N, C = 2, 8192, 256
S = 2048
NCORES = 8
SCALE = 0.0625
NEG = -1.0e9

XA, XB, XC = 2048, 1024, 1536          # slab section widths
XW = XA + XB + XC                      # 4608

_PROG = None


def _build_program():
    import concourse.mybir as mybir
    import concourse.tile as tile
    from concourse import bacc

    F32 = mybir.dt.float32
    BF16 = mybir.dt.bfloat16
    FP8 = mybir.dt.float8e4
    DR = mybir.MatmulPerfMode.DoubleRow
    Exp = mybir.ActivationFunctionType.Exp

    nc = bacc.Bacc("TRN2", target_bir_lowering=False, debug=False,
                   num_devices=NCORES)

    x_d = nc.dram_tensor("x", [128, 2 * XW], BF16, kind="ExternalInput")
    w_d = nc.dram_tensor("w", [128, 2 * 3 * C], BF16, kind="ExternalInput")
    bias_d = nc.dram_tensor("bias", [128, 5], F32, kind="ExternalInput")
    out_d = nc.dram_tensor("out", [S, C], F32, kind="ExternalOutput")

    with tile.TileContext(nc) as tc:
        with (
            tc.tile_pool(name="const", bufs=1) as cpool,
            tc.tile_pool(name="xsb", bufs=1) as xpool,
            tc.tile_pool(name="kqt", bufs=1) as kqpool,
            tc.tile_pool(name="vext", bufs=32) as vpool,
            tc.tile_pool(name="probs", bufs=8) as ppool,
            tc.tile_pool(name="probs2", bufs=8) as ppool2,
            tc.tile_pool(name="probs8", bufs=8) as ppool8,
            tc.tile_pool(name="stage", bufs=12) as spool,
            tc.tile_pool(name="fin", bufs=4) as fpool,
            tc.tile_pool(name="ps_s", bufs=3, space="PSUM") as ps_scores,
            tc.tile_pool(name="ps_u", bufs=2, space="PSUM") as ps_small,
        ):
            # ---- weights + early x chunks first (PE warmup) ----
            w_sb = cpool.tile([128, 2, 3 * C], BF16, tag="w")
            nc.sync.dma_start(w_sb[:], w_d.rearrange("p (c k) -> p c k", c=2))

            xb = xpool.tile([128, 2, XW], BF16, tag="xb")
            x_r = x_d.rearrange("p (c k) -> p c k", c=2)
            CH = 512
            CHUNK_ORDER = [0, 1, 6, 7, 8, 2, 3, 4, 5]
            for i, ch in enumerate(CHUNK_ORDER):
                eng = nc.sync if i % 2 == 0 else nc.gpsimd
                eng.dma_start(xb[:, :, CH * ch:CH * (ch + 1)],
                              x_r[:, :, CH * ch:CH * (ch + 1)])
            bias_t = cpool.tile([128, 5], F32, tag="bias")
            nc.sync.dma_start(bias_t[:], bias_d[:])

            def wap(name, ci, co):
                i = {"q": 0, "k": 1, "v": 2}[name]
                return w_sb[:, ci, 256 * i + 128 * co:256 * i + 128 * (co + 1)]

            # ---- constants ----
            ones_t = cpool.tile([128, 128], F32, tag="ones")
            nc.gpsimd.memset(ones_t[:], 1.0)
            # identity (stationary) and -1e9 strict-upper (moving): adding
            # ident.T @ trineg to a diag score tile applies the causal mask
            # inside the PSUM accumulation group, PE-side.
            id_f = cpool.tile([128, 128], F32, tag="id_f")
            nc.gpsimd.affine_select(
                out=id_f[:], in_=ones_t[:],
                compare_op=mybir.AluOpType.is_equal,
                fill=0.0, base=0, pattern=[[1, 128]], channel_multiplier=-1)
            ident = cpool.tile([128, 128], BF16, tag="ident")
            nc.vector.tensor_copy(ident[:], id_f[:])
            neg_t = cpool.tile([128, 128], F32, tag="negs")
            nc.gpsimd.memset(neg_t[:], NEG)
            tri_f = cpool.tile([128, 128], F32, tag="tri_f")
            # keep NEG where p > f (mask keys k > query q), 0 elsewhere
            nc.gpsimd.affine_select(
                out=tri_f[:], in_=neg_t[:],
                compare_op=mybir.AluOpType.is_ge,
                fill=0.0, base=-1, pattern=[[-1, 128]], channel_multiplier=1)
            trineg = cpool.tile([128, 128], BF16, tag="trineg")
            nc.vector.tensor_copy(trineg[:], tri_f[:])

            pmats = {}
            for stride, u in [(2, 0), (2, 1), (4, 0), (4, 1), (4, 2), (4, 3)]:
                pf = cpool.tile([128, 128], F32, tag="fscratch",
                                name=f"pmf{stride}_{u}")
                nc.gpsimd.affine_select(
                    out=pf[:], in_=ones_t[:],
                    compare_op=mybir.AluOpType.is_equal,
                    fill=0.0, base=128 * u,
                    pattern=[[1, 128]], channel_multiplier=-stride)
                pm = cpool.tile([128, 128], BF16, tag=f"pm{stride}_{u}")
                nc.vector.tensor_copy(pm[:], pf[:])
                pmats[stride, u] = pm

            # ---- projection helpers (emitted lazily, in job-driven order) --
            ktb = kqpool.tile([128, 2, XA], BF16, tag="ktb")
            qtb = kqpool.tile([128, 2, XA], BF16, tag="qtb")
            kt8 = kqpool.tile([128, 2, XB + XC], FP8, tag="kt8")
            qt8 = kqpool.tile([128, 2, 1536], FP8, tag="qt8")
            kt8a = kqpool.tile([128, 2, XA], FP8, tag="kt8a")
            qt8a = kqpool.tile([128, 2, XA], FP8, tag="qt8a")
            _cp_flip = [0]

            def copy_to(dst, src):
                _cp_flip[0] ^= 1
                if _cp_flip[0]:
                    nc.scalar.copy(dst, src)
                else:
                    nc.vector.tensor_copy(dst, src)

            def kproj(ch):
                c0 = CH * ch
                for co in range(2):
                    ps = ps_small.tile([128, 512], F32, tag="proj", bufs=3)
                    for ci in range(2):
                        nc.tensor.matmul(
                            ps[:], wap("k", ci, co), xb[:, ci, c0:c0 + CH],
                            start=(ci == 0), stop=(ci == 1))
                    if c0 >= XA:
                        copy_to(kt8[:, co, c0 - XA:c0 - XA + CH], ps[:])
                    else:
                        copy_to(ktb[:, co, c0:c0 + CH], ps[:])

            def qproj(ch):
                c0 = CH * ch
                for co in range(2):
                    ps = ps_small.tile([128, 512], F32, tag="proj", bufs=3)
                    for ci in range(2):
                        nc.tensor.matmul(
                            ps[:], wap("q", ci, co), xb[:, ci, c0:c0 + CH],
                            start=(ci == 0), stop=(ci == 1))
                    copy_to(qtb[:, co, c0:c0 + CH], ps[:])
                    nc.gpsimd.tensor_copy(
                        qt8[:, co, c0 // 2:c0 // 2 + 256],
                        qtb[:, co, c0:c0 + CH:2])
                    nc.gpsimd.tensor_copy(
                        qt8[:, co, 1024 + c0 // 4:1024 + c0 // 4 + 128],
                        qtb[:, co, c0:c0 + CH:4])

            def vproj_pair(src_c0, name, dt=BF16):
                """project x cols [src_c0, src_c0+256) -> one [128,2,258]"""
                ps = ps_small.tile([128, 512], F32, tag="proj", bufs=3,
                                   name=f"psv{name}")
                for half in range(2):
                    for ci in range(2):
                        nc.tensor.matmul(
                            ps[:, 256 * half:256 * (half + 1)],
                            xb[:, ci,
                               src_c0 + 128 * half:src_c0 + 128 * (half + 1)],
                            w_sb[:, ci, 512:768],
                            start=(ci == 0), stop=(ci == 1))
                v = vpool.tile([128, 2, 258], dt, tag="vp", name=f"vp{name}")
                if dt is BF16:
                    copy_to(v[:, :, 0:256],
                            ps.rearrange("p (h c) -> p h c", h=2))
                else:
                    copy_to(v[:, :, 0:256],
                            ps.rearrange("p (h c) -> p h c", h=2))
                nc.vector.memset(v[:, :, 256:257], 1.0)
                nc.vector.memset(v[:, :, 257:258], 0.0)
                return v

            vA = [None] * 8
            vA8 = [None] * 8

            def vAp(j):
                if vA[j] is None:
                    vA[j] = vproj_pair(256 * j, f"A{j}")
                return vA[j]

            def vBd_gather(j):
                """B diag pair j rows = A rows [512j, 512j+512) step 2."""
                v = vpool.tile([128, 2, 258], BF16, tag="vp", name=f"vBd{j}")
                for half in range(2):
                    for s in range(2):
                        src = vAp(2 * j + half)
                        nc.sync.dma_start(
                            v[64 * s:64 * (s + 1), half, :],
                            src[0:128:2, s, :])
                return v

            def vCd_gather(j):
                """C diag pair j rows = A rows [1024j, 1024j+1024) step 4."""
                v = vpool.tile([128, 2, 258], BF16, tag="vp", name=f"vCd{j}")
                for half in range(2):
                    base = 512 * (2 * j + half)
                    for s in range(4):
                        src = vAp((base + 128 * s) // 256)
                        sh = ((base + 128 * s) // 128) % 2
                        nc.sync.dma_start(
                            v[32 * s:32 * (s + 1), half, :],
                            src[0:128:4, sh, :])
                return v

            jobs = {
                "C": dict(nq=4, P=12, stride=4, kpre=XA + XB, dstep=4),
                "B": dict(nq=8, P=8, stride=2, kpre=XA, dstep=2),
                "A": dict(nq=16, P=0, stride=1, kpre=None, dstep=1),
            }
            vslabs = {}

            def kstat(job, ci, kt):
                j = jobs[job]
                if kt < j["P"]:
                    c0 = j["kpre"] + 128 * kt
                    return ktb[:, ci, c0:c0 + 128]
                d = kt - j["P"]
                st = j["dstep"]
                return ktb[:, ci, 128 * st * d:128 * st * (d + 1):st]

            def qmov(job, ci, tau):
                st = jobs[job]["dstep"]
                return qtb[:, ci, 128 * st * tau:128 * st * (tau + 1):st]

            def kstat8(job, kt):
                if job == "A":
                    return kt8a[:, :, 128 * kt:128 * (kt + 1)]
                off = 0 if job == "B" else XB
                return kt8[:, :, off + 128 * kt:off + 128 * (kt + 1)]

            def qmov8(job, tau):
                if job == "A":
                    return qt8a[:, :, 128 * tau:128 * (tau + 1)]
                off = 0 if job == "B" else 1024
                return qt8[:, :, off + 128 * tau:off + 128 * (tau + 1)]

            def vmov(job, kt):
                j = vslabs[job][kt // 2]
                return j[:, kt % 2, :]

            st_tiles = {}

            def do_qtile(job, tau, ups):
                """scores+exp+U for q-tile tau of job; U accumulated into
                psum tile `ups` (A: group already started by placements)."""
                j = jobs[job]
                P = j["P"]
                nkt = P + tau + 1
                pbs = []            # (pb_tile, kt0, nkt_in_group, is8)
                for gi, g0 in enumerate(range(0, nkt, 4)):
                    gn = min(4, nkt - g0)
                    # groups fully inside the prefix use the fp8 DoubleRow
                    # path (long-range diffuse attention tolerates fp8)
                    if job == "A":
                        is8 = g0 + gn - 1 <= tau - 5
                    else:
                        is8 = g0 + gn <= P
                    ps = ps_scores.tile([128, 512], F32, tag="scores")
                    if is8:
                        for i in range(gn):
                            kt = g0 + i
                            nc.tensor.matmul(
                                ps[:, 128 * i:128 * (i + 1)],
                                kstat8(job, kt), qmov8(job, tau),
                                start=True, stop=True, perf_mode=DR)
                        if job == "A":
                            bias_ap = bias_t[:, 4:5]
                        else:
                            b0 = 0 if job == "B" else 1 + g0 // 4
                            bias_ap = bias_t[:, b0:b0 + 1]
                        pb = ppool8.tile([128, 512], FP8, tag="pb8")
                    else:
                        for i in range(gn):
                            kt = g0 + i
                            diag = kt == nkt - 1
                            reg = ps[:, 128 * i:128 * (i + 1)]
                            for ci in range(2):
                                nc.tensor.matmul(
                                    reg, kstat(job, ci, kt),
                                    qmov(job, ci, tau),
                                    start=(ci == 0),
                                    stop=(ci == 1 and not diag))
                            if diag:
                                nc.tensor.matmul(reg, ident[:], trineg[:],
                                                 start=False, stop=True)
                        if g0 < P:
                            b0 = 0 if job == "B" else 1 + g0 // 4
                            bias_ap = bias_t[:, b0:b0 + 1]
                        else:
                            bias_ap = bias_t[:, 4:5]
                        pb = (ppool if gi % 2 == 0 else ppool2).tile(
                            [128, 512], BF16, tag="pb")
                    pbs.append((pb, g0, gn, is8))
                    nc.scalar.activation(pb[:, 0:128 * gn], ps[:, 0:128 * gn],
                                         Exp, bias=bias_ap, scale=SCALE)
                for pb, g0, gn, is8 in pbs:
                    if is8:
                        for jj in range(gn // 2):
                            kt = g0 + 2 * jj
                            nc.tensor.matmul(
                                ups[:],
                                pb[:, 256 * jj:256 * (jj + 1)].rearrange(
                                    "p (two f) -> p two f", two=2),
                                (vA8 if job == "A"
                                 else vslabs[job])[kt // 2][:],
                                start=(kt == 0 and job != "A"),
                                stop=False, perf_mode=DR)
                    else:
                        for i in range(gn):
                            kt = g0 + i
                            nc.tensor.matmul(
                                ups[:], pb[:, 128 * i:128 * (i + 1)],
                                vmov(job, kt),
                                start=(kt == 0 and job != "A"),
                                stop=(kt == nkt - 1))

            def run_shared(job, tau):
                ups = ps_small.tile([128, 258], F32, tag="u",
                                    name=f"u{job}{tau}")[:]
                do_qtile(job, tau, ups)
                st = spool.tile([128, 258], BF16, tag="st",
                                name=f"st{job}{tau}")
                nc.vector.tensor_copy(st[:], ups[:])
                st_tiles[job, tau] = st

            # ================= emission schedule =================
            # projections needed for C first, C starts ASAP, then B, then A.
            kproj(0); qproj(0)
            vAp(0); vAp(1)
            kproj(1); qproj(1)
            vAp(2); vAp(3)
            for ch in (6, 7, 8):
                kproj(ch)
            vCpre = [vproj_pair(XA + XB + 256 * j, f"Cp{j}", FP8) for j in range(6)]
            vslabC = vCpre + [vCd_gather(0)]
            vslabs["C"] = vslabC           # vCd[1] appended below

            run_shared("C", 0)
            kproj(2); qproj(2)
            vAp(4); vAp(5)
            run_shared("C", 1)
            kproj(3); qproj(3)
            vAp(6); vAp(7)
            vslabC.append(vCd_gather(1))
            run_shared("C", 2)
            kproj(4); kproj(5)
            vBpre = [vproj_pair(XA + 256 * j, f"Bp{j}", FP8) for j in range(4)]
            run_shared("C", 3)
            vBd = [vBd_gather(j) for j in range(4)]
            vslabs["B"] = vBpre + vBd
            vslabs["A"] = vA

            # ---- B job interleaved with A (A tile t needs st_B[t//2]) ----
            fin = [fpool.tile([128, 2, 256], F32, bufs=1, tag=f"fing{g}",
                              name=f"fing{g}") for g in range(8)]
            out_r = out_d.rearrange("(g t p) c -> g p t c", p=128, t=2)

            def run_A(t):
                ups = ps_small.tile([128, 258], F32, tag="u",
                                    name=f"uA{t}")[:]
                nc.tensor.matmul(ups, pmats[4, t % 4][:],
                                 st_tiles["C", t // 4][:],
                                 start=True, stop=False)
                nc.tensor.matmul(ups, pmats[2, t % 2][:],
                                 st_tiles["B", t // 2][:],
                                 start=False, stop=False)
                do_qtile("A", t, ups)
                g, ti = divmod(t, 2)
                rec = fpool.tile([128, 1], F32, tag="rec")
                nc.vector.reciprocal(rec[:], ups[:, 256:257])
                nc.vector.tensor_scalar_mul(fin[g][:, ti, :],
                                            ups[:, 0:256], rec[:])
                if ti == 1:
                    eng = nc.sync if g % 2 == 0 else nc.gpsimd
                    eng.dma_start(out_r[g], fin[g][:])

            # deferred fp8 twins of A-region K/Q/V (used by far A tiles,
            # first at A t=8 -> spread emission across early B iterations)
            def emit_twins(part):
                for ch in (part, part + 2):
                    for co in range(2):
                        c0 = CH * ch
                        nc.gpsimd.tensor_copy(kt8a[:, co, c0:c0 + CH],
                                              ktb[:, co, c0:c0 + CH])
                        nc.gpsimd.tensor_copy(qt8a[:, co, c0:c0 + CH],
                                              qtb[:, co, c0:c0 + CH])
                for j in (4 * part, 4 * part + 1, 4 * part + 2, 4 * part + 3):
                    v8 = vpool.tile([128, 2, 258], FP8, tag="vp",
                                    name=f"vA8_{j}")
                    nc.gpsimd.tensor_copy(v8[:], vA[j][:])
                    vA8[j] = v8

            for tau in range(8):
                run_shared("B", tau)
                if tau < 2:
                    emit_twins(tau)
                run_A(2 * tau)
                run_A(2 * tau + 1)

    nc.compile()
    return nc


def _get_program():
    global _PROG
    if _PROG is None:
        _PROG = _build_program()
    return _PROG


def make_in_maps(x, Wq, Wk, Wv):
    """Host-side sharding: gather / transpose / zero-pad / dtype cast only."""
    import ml_dtypes
    bf16 = np.dtype(ml_dtypes.bfloat16)
    x = np.asarray(x, dtype=np.float32)
    w_all = np.concatenate([np.asarray(Wq, np.float32),
                            np.asarray(Wk, np.float32),
                            np.asarray(Wv, np.float32)], axis=1)  # [C, 3C]
    w_t = np.ascontiguousarray(
        w_all.reshape(2, 128, 3 * C).transpose(1, 0, 2).reshape(128, 6 * C)
    ).astype(bf16)

    in_maps = []
    for d in range(NCORES):
        b, q = divmod(d, 4)
        quarter = x[b, 2048 * q:2048 * (q + 1), :]          # [2048, C]

        seg = 0 if q < 2 else 4096
        grp2 = x[b, seg:seg + 4096:2, :]                    # [2048, C]
        r0 = 1024 * (q % 2)
        bpre = grp2[0:1024] if r0 == 1024 else np.zeros((XB, C), np.float32)

        grp4 = x[b, 0:8192:4, :]                            # [2048, C]
        r0c = 512 * q
        cpre = np.concatenate(
            [grp4[0:r0c], np.zeros((XC - r0c, C), np.float32)], axis=0)

        slab = np.concatenate([quarter, bpre, cpre], axis=0)  # [XW, C]
        xt = np.ascontiguousarray(
            slab.T.reshape(2, 128, XW).transpose(1, 0, 2).reshape(128, 2 * XW)
        ).astype(bf16)

        bias = np.zeros((128, 5), np.float32)
        bias[:, 4] = -2.0
        bias[:, 0] = -2.0 if r0 == 1024 else NEG
        for g in range(3):
            bias[:, 1 + g] = -2.0 if g < q else NEG

        in_maps.append({"x": xt, "w": w_t, "bias": bias})
    return in_maps


def kernel(x, Wq, Wk, Wv):
    from concourse.bass_utils import run_bass_kernel_spmd

    nc = _get_program()
    in_maps = make_in_maps(x, Wq, Wk, Wv)
    res = run_bass_kernel_spmd(nc, in_maps, core_ids=list(range(NCORES)))
    out = np.empty((B, N, C), np.float32)
    for d in range(NCORES):
        b, q = divmod(d, 4)
        out[b, 2048 * q:2048 * (q + 1), :] = res.results[d]["out"]
    return out


# revision 12
# speedup vs baseline: 1.3948x; 1.0060x over previous
"""Dilated self-attention TRN2 Bass kernel, v2 (bf16, exact causal shapes).

Problem (hardcoded): B=2, N=8192, C=256, WS=[2048,4096,8192], RS=[1,2,4],
HEAD_IDX=0 -> G=7 groups of s=2048 rows each.

Sharding: core d = (b=d//4, q=d%4) owns output positions [2048q, 2048(q+1))
of batch b.  Jobs per core (all bf16 matmuls, 1 cycle/row at any width):
  C: 4 q-tiles (slots 12..15) of the r=4 group, prefix 12 kt (data-masked)
  B: 8 q-tiles (slots 8..15) of the r=2 half, prefix 8 kt (data-masked)
  A: 16 q-tiles full causal over own quarter
Layout tricks vs v1:
  - One bf16 x slab [128,2,4608] = [A 2048 | Bpre 1024 | Cpre 1536]; B/C
    diag K/Q are strided views into the A region (no re-projection).
  - V tiles for B/C diag are partition-gathered from A's V tiles via
    SBUF->SBUF DMA (stride 2/4) instead of re-projecting.
  - Exact per-q-tile causal prefixes (no 512-block rounding).
  - Scatter-add runs inside the output PSUM accumulation group: placement
    matmuls for B/C land before A's U matmuls; finalize reads PSUM once.
  - Emission order starts the C job while A/B projections still run.
"""

import numpy as np

B, N, C = 2, 8192, 256
S = 2048
NCORES = 8
SCALE = 0.0625
NEG = -1.0e9

XA, XB, XC = 2048, 1024, 1536          # slab section widths
XW = XA + XB + XC                      # 4608

_PROG = None


def _build_program():
    import concourse.mybir as mybir
    import concourse.tile as tile
    from concourse import bacc

    F32 = mybir.dt.float32
    BF16 = mybir.dt.bfloat16
    FP8 = mybir.dt.float8e4
    DR = mybir.MatmulPerfMode.DoubleRow
    Exp = mybir.ActivationFunctionType.Exp

    nc = bacc.Bacc("TRN2", target_bir_lowering=False, debug=False,
                   num_devices=NCORES)

    x_d = nc.dram_tensor("x", [128, 2 * XW], BF16, kind="ExternalInput")
    w_d = nc.dram_tensor("w", [128, 2 * 3 * C], BF16, kind="ExternalInput")
    bias_d = nc.dram_tensor("bias", [128, 5], F32, kind="ExternalInput")
    out_d = nc.dram_tensor("out", [S, C], F32, kind="ExternalOutput")

    with tile.TileContext(nc) as tc:
        with (
            tc.tile_pool(name="const", bufs=1) as cpool,
            tc.tile_pool(name="xsb", bufs=1) as xpool,
            tc.tile_pool(name="kqt", bufs=1) as kqpool,
            tc.tile_pool(name="vext", bufs=32) as vpool,
            tc.tile_pool(name="probs", bufs=8) as ppool,
            tc.tile_pool(name="probs2", bufs=8) as ppool2,
            tc.tile_pool(name="probs8", bufs=8) as ppool8,
            tc.tile_pool(name="stage", bufs=12) as spool,
            tc.tile_pool(name="fin", bufs=4) as fpool,
            tc.tile_pool(name="ps_s", bufs=3, space="PSUM") as ps_scores,
            tc.tile_pool(name="ps_u", bufs=2, space="PSUM") as ps_small,
        ):
            # ---- weights + early x chunks first (PE warmup) ----
            w_sb = cpool.tile([128, 2, 3 * C], BF16, tag="w")
            nc.sync.dma_start(w_sb[:], w_d.rearrange("p (c k) -> p c k", c=2))

            xb = xpool.tile([128, 2, XW], BF16, tag="xb")
            x_r = x_d.rearrange("p (c k) -> p c k", c=2)
            CH = 512
            CHUNK_ORDER = [0, 1, 6, 7, 8, 2, 3, 4, 5]
            for i, ch in enumerate(CHUNK_ORDER):
                eng = nc.sync if i % 2 == 0 else nc.gpsimd
                eng.dma_start(xb[:, :, CH * ch:CH * (ch + 1)],
                              x_r[:, :, CH * ch:CH * (ch + 1)])
            bias_t = cpool.tile([128, 5], F32, tag="bias")
            nc.sync.dma_start(bias_t[:], bias_d[:])

            def wap(name, ci, co):
                i = {"q": 0, "k": 1, "v": 2}[name]
                return w_sb[:, ci, 256 * i + 128 * co:256 * i + 128 * (co + 1)]

            # ---- constants ----
            ones_t = cpool.tile([128, 128], F32, tag="ones")
            nc.gpsimd.memset(ones_t[:], 1.0)
            # identity (stationary) and -1e9 strict-upper (moving): adding
            # ident.T @ trineg to a diag score tile applies the causal mask
            # inside the PSUM accumulation group, PE-side.
            id_f = cpool.tile([128, 128], F32, tag="id_f")
            nc.gpsimd.affine_select(
                out=id_f[:], in_=ones_t[:],
                compare_op=mybir.AluOpType.is_equal,
                fill=0.0, base=0, pattern=[[1, 128]], channel_multiplier=-1)
            ident = cpool.tile([128, 128], BF16, tag="ident")
            nc.vector.tensor_copy(ident[:], id_f[:])
            neg_t = cpool.tile([128, 128], F32, tag="negs")
            nc.gpsimd.memset(neg_t[:], NEG)
            tri_f = cpool.tile([128, 128], F32, tag="tri_f")
            # keep NEG where p > f (mask keys k > query q), 0 elsewhere
            nc.gpsimd.affine_select(
                out=tri_f[:], in_=neg_t[:],
                compare_op=mybir.AluOpType.is_ge,
                fill=0.0, base=-1, pattern=[[-1, 128]], channel_multiplier=1)
            trineg = cpool.tile([128, 128], BF16, tag="trineg")
            nc.vector.tensor_copy(trineg[:], tri_f[:])

            pmats = {}

            def build_pmats():
                for stride, u in [(2, 0), (2, 1), (4, 0), (4, 1),
                                  (4, 2), (4, 3)]:
                    pf = cpool.tile([128, 128], F32, tag="fscratch",
                                    name=f"pmf{stride}_{u}")
                    nc.gpsimd.affine_select(
                        out=pf[:], in_=ones_t[:],
                        compare_op=mybir.AluOpType.is_equal,
                        fill=0.0, base=128 * u,
                        pattern=[[1, 128]], channel_multiplier=-stride)
                    pm = cpool.tile([128, 128], BF16, tag=f"pm{stride}_{u}")
                    nc.vector.tensor_copy(pm[:], pf[:])
                    pmats[stride, u] = pm

            # ---- projection helpers (emitted lazily, in job-driven order) --
            ktb = kqpool.tile([128, 2, XA], BF16, tag="ktb")
            qtb = kqpool.tile([128, 2, XA], BF16, tag="qtb")
            kt8 = kqpool.tile([128, 2, XB + XC], FP8, tag="kt8")
            qt8 = kqpool.tile([128, 2, 1536], FP8, tag="qt8")
            kt8a = kqpool.tile([128, 2, XA], FP8, tag="kt8a")
            qt8a = kqpool.tile([128, 2, XA], FP8, tag="qt8a")
            _cp_flip = [0]

            def copy_to(dst, src):
                _cp_flip[0] ^= 1
                if _cp_flip[0]:
                    nc.scalar.copy(dst, src)
                else:
                    nc.vector.tensor_copy(dst, src)

            def kproj(ch):
                c0 = CH * ch
                for co in range(2):
                    ps = ps_small.tile([128, 512], F32, tag="proj", bufs=3)
                    for ci in range(2):
                        nc.tensor.matmul(
                            ps[:], wap("k", ci, co), xb[:, ci, c0:c0 + CH],
                            start=(ci == 0), stop=(ci == 1))
                    if c0 >= XA:
                        copy_to(kt8[:, co, c0 - XA:c0 - XA + CH], ps[:])
                    else:
                        copy_to(ktb[:, co, c0:c0 + CH], ps[:])

            def qproj(ch):
                c0 = CH * ch
                for co in range(2):
                    ps = ps_small.tile([128, 512], F32, tag="proj", bufs=3)
                    for ci in range(2):
                        nc.tensor.matmul(
                            ps[:], wap("q", ci, co), xb[:, ci, c0:c0 + CH],
                            start=(ci == 0), stop=(ci == 1))
                    copy_to(qtb[:, co, c0:c0 + CH], ps[:])
                    nc.gpsimd.tensor_copy(
                        qt8[:, co, c0 // 2:c0 // 2 + 256],
                        qtb[:, co, c0:c0 + CH:2])
                    nc.gpsimd.tensor_copy(
                        qt8[:, co, 1024 + c0 // 4:1024 + c0 // 4 + 128],
                        qtb[:, co, c0:c0 + CH:4])

            def vproj_pair(src_c0, name, dt=BF16):
                """project x cols [src_c0, src_c0+256) -> one [128,2,258]"""
                ps = ps_small.tile([128, 512], F32, tag="proj", bufs=3,
                                   name=f"psv{name}")
                for half in range(2):
                    for ci in range(2):
                        nc.tensor.matmul(
                            ps[:, 256 * half:256 * (half + 1)],
                            xb[:, ci,
                               src_c0 + 128 * half:src_c0 + 128 * (half + 1)],
                            w_sb[:, ci, 512:768],
                            start=(ci == 0), stop=(ci == 1))
                v = vpool.tile([128, 2, 258], dt, tag="vp", name=f"vp{name}")
                if dt is BF16:
                    copy_to(v[:, :, 0:256],
                            ps.rearrange("p (h c) -> p h c", h=2))
                else:
                    copy_to(v[:, :, 0:256],
                            ps.rearrange("p (h c) -> p h c", h=2))
                nc.vector.memset(v[:, :, 256:257], 1.0)
                nc.vector.memset(v[:, :, 257:258], 0.0)
                return v

            vA = [None] * 8
            vA8 = [None] * 8

            def vAp(j):
                if vA[j] is None:
                    vA[j] = vproj_pair(256 * j, f"A{j}")
                return vA[j]

            def vBd_gather(j):
                """B diag pair j rows = A rows [512j, 512j+512) step 2."""
                v = vpool.tile([128, 2, 258], BF16, tag="vp", name=f"vBd{j}")
                for half in range(2):
                    for s in range(2):
                        src = vAp(2 * j + half)
                        nc.sync.dma_start(
                            v[64 * s:64 * (s + 1), half, :],
                            src[0:128:2, s, :])
                return v

            def vCd_gather(j):
                """C diag pair j rows = A rows [1024j, 1024j+1024) step 4."""
                v = vpool.tile([128, 2, 258], BF16, tag="vp", name=f"vCd{j}")
                for half in range(2):
                    base = 512 * (2 * j + half)
                    for s in range(4):
                        src = vAp((base + 128 * s) // 256)
                        sh = ((base + 128 * s) // 128) % 2
                        nc.sync.dma_start(
                            v[32 * s:32 * (s + 1), half, :],
                            src[0:128:4, sh, :])
                return v

            jobs = {
                "C": dict(nq=4, P=12, stride=4, kpre=XA + XB, dstep=4),
                "B": dict(nq=8, P=8, stride=2, kpre=XA, dstep=2),
                "A": dict(nq=16, P=0, stride=1, kpre=None, dstep=1),
            }
            vslabs = {}

            def kstat(job, ci, kt):
                j = jobs[job]
                if kt < j["P"]:
                    c0 = j["kpre"] + 128 * kt
                    return ktb[:, ci, c0:c0 + 128]
                d = kt - j["P"]
                st = j["dstep"]
                return ktb[:, ci, 128 * st * d:128 * st * (d + 1):st]

            def qmov(job, ci, tau):
                st = jobs[job]["dstep"]
                return qtb[:, ci, 128 * st * tau:128 * st * (tau + 1):st]

            def kstat8(job, kt):
                if job == "A":
                    return kt8a[:, :, 128 * kt:128 * (kt + 1)]
                off = 0 if job == "B" else XB
                return kt8[:, :, off + 128 * kt:off + 128 * (kt + 1)]

            def qmov8(job, tau):
                if job == "A":
                    return qt8a[:, :, 128 * tau:128 * (tau + 1)]
                off = 0 if job == "B" else 1024
                return qt8[:, :, off + 128 * tau:off + 128 * (tau + 1)]

            def vmov(job, kt):
                j = vslabs[job][kt // 2]
                return j[:, kt % 2, :]

            st_tiles = {}

            def do_qtile(job, tau, ups):
                """scores+exp+U for q-tile tau of job; U accumulated into
                psum tile `ups` (A: group already started by placements)."""
                j = jobs[job]
                P = j["P"]
                nkt = P + tau + 1
                pbs = []            # (pb_tile, kt0, nkt_in_group, is8)
                for gi, g0 in enumerate(range(0, nkt, 4)):
                    gn = min(4, nkt - g0)
                    # groups fully inside the prefix use the fp8 DoubleRow
                    # path (long-range diffuse attention tolerates fp8)
                    if job == "A":
                        is8 = g0 + gn - 1 <= tau - 5
                    else:
                        is8 = g0 + gn <= P
                    ps = ps_scores.tile([128, 512], F32, tag="scores")
                    if is8:
                        for i in range(gn):
                            kt = g0 + i
                            nc.tensor.matmul(
                                ps[:, 128 * i:128 * (i + 1)],
                                kstat8(job, kt), qmov8(job, tau),
                                start=True, stop=True, perf_mode=DR)
                        if job == "A":
                            bias_ap = bias_t[:, 4:5]
                        else:
                            b0 = 0 if job == "B" else 1 + g0 // 4
                            bias_ap = bias_t[:, b0:b0 + 1]
                        pb = ppool8.tile([128, 512], FP8, tag="pb8")
                    else:
                        for i in range(gn):
                            kt = g0 + i
                            diag = kt == nkt - 1
                            reg = ps[:, 128 * i:128 * (i + 1)]
                            for ci in range(2):
                                nc.tensor.matmul(
                                    reg, kstat(job, ci, kt),
                                    qmov(job, ci, tau),
                                    start=(ci == 0),
                                    stop=(ci == 1 and not diag))
                            if diag:
                                nc.tensor.matmul(reg, ident[:], trineg[:],
                                                 start=False, stop=True)
                        if g0 < P:
                            b0 = 0 if job == "B" else 1 + g0 // 4
                            bias_ap = bias_t[:, b0:b0 + 1]
                        else:
                            bias_ap = bias_t[:, 4:5]
                        pb = (ppool if gi % 2 == 0 else ppool2).tile(
                            [128, 512], BF16, tag="pb")
                    pbs.append((pb, g0, gn, is8))
                    nc.scalar.activation(pb[:, 0:128 * gn], ps[:, 0:128 * gn],
                                         Exp, bias=bias_ap, scale=SCALE)
                for pb, g0, gn, is8 in pbs:
                    if is8:
                        for jj in range(gn // 2):
                            kt = g0 + 2 * jj
                            nc.tensor.matmul(
                                ups[:],
                                pb[:, 256 * jj:256 * (jj + 1)].rearrange(
                                    "p (two f) -> p two f", two=2),
                                (vA8 if job == "A"
                                 else vslabs[job])[kt // 2][:],
                                start=(kt == 0 and job != "A"),
                                stop=False, perf_mode=DR)
                    else:
                        for i in range(gn):
                            kt = g0 + i
                            nc.tensor.matmul(
                                ups[:], pb[:, 128 * i:128 * (i + 1)],
                                vmov(job, kt),
                                start=(kt == 0 and job != "A"),
                                stop=(kt == nkt - 1))

            def run_shared(job, tau):
                ups = ps_small.tile([128, 258], F32, tag="u",
                                    name=f"u{job}{tau}")[:]
                do_qtile(job, tau, ups)
                st = spool.tile([128, 258], BF16, tag="st",
                                name=f"st{job}{tau}")
                nc.vector.tensor_copy(st[:], ups[:])
                st_tiles[job, tau] = st

            # ================= emission schedule =================
            # projections needed for C first, C starts ASAP, then B, then A.
            kproj(0); qproj(0)
            vAp(0); vAp(1)
            kproj(1); qproj(1)
            vAp(2); vAp(3)
            for ch in (6, 7, 8):
                kproj(ch)
            vCpre = [vproj_pair(XA + XB + 256 * j, f"Cp{j}", FP8) for j in range(6)]
            vslabC = vCpre + [vCd_gather(0)]
            vslabs["C"] = vslabC           # vCd[1] appended below

            run_shared("C", 0)
            kproj(2); qproj(2)
            vAp(4); vAp(5)
            run_shared("C", 1)
            kproj(3); qproj(3)
            vAp(6); vAp(7)
            vslabC.append(vCd_gather(1))
            run_shared("C", 2)
            kproj(4); kproj(5)
            vBpre = [vproj_pair(XA + 256 * j, f"Bp{j}", FP8) for j in range(4)]
            run_shared("C", 3)
            vBd = [vBd_gather(j) for j in range(4)]
            vslabs["B"] = vBpre + vBd
            vslabs["A"] = vA
            build_pmats()

            # ---- B job interleaved with A (A tile t needs st_B[t//2]) ----
            out_r = out_d.rearrange("(t p) c -> t p c", p=128)

            def run_A(t):
                ups = ps_small.tile([128, 258], F32, tag="u",
                                    name=f"uA{t}")[:]
                nc.tensor.matmul(ups, pmats[4, t % 4][:],
                                 st_tiles["C", t // 4][:],
                                 start=True, stop=False)
                nc.tensor.matmul(ups, pmats[2, t % 2][:],
                                 st_tiles["B", t // 2][:],
                                 start=False, stop=False)
                do_qtile("A", t, ups)
                rec = fpool.tile([128, 1], F32, tag="rec")
                nc.vector.reciprocal(rec[:], ups[:, 256:257])
                ft = fpool.tile([128, 256], F32, tag="fin", bufs=4,
                                name=f"fin{t}")
                nc.vector.tensor_scalar_mul(ft[:], ups[:, 0:256], rec[:])
                eng = nc.sync if t % 2 == 0 else nc.gpsimd
                eng.dma_start(out_r[t], ft[:])

            # deferred fp8 twins of A-region K/Q/V (used by far A tiles,
            # first at A t=8 -> spread emission across early B iterations)
            def emit_twins(part):
                for ch in (part, part + 2):
                    for co in range(2):
                        c0 = CH * ch
                        nc.gpsimd.tensor_copy(kt8a[:, co, c0:c0 + CH],
                                              ktb[:, co, c0:c0 + CH])
                        nc.gpsimd.tensor_copy(qt8a[:, co, c0:c0 + CH],
                                              qtb[:, co, c0:c0 + CH])
                for j in (4 * part, 4 * part + 1, 4 * part + 2, 4 * part + 3):
                    v8 = vpool.tile([128, 2, 258], FP8, tag="vp",
                                    name=f"vA8_{j}")
                    nc.gpsimd.tensor_copy(v8[:], vA[j][:])
                    vA8[j] = v8

            for tau in range(8):
                run_shared("B", tau)
                if tau < 2:
                    emit_twins(tau)
                run_A(2 * tau)
                run_A(2 * tau + 1)

    nc.compile()
    return nc


def _get_program():
    global _PROG
    if _PROG is None:
        _PROG = _build_program()
    return _PROG


def make_in_maps(x, Wq, Wk, Wv):
    """Host-side sharding: gather / transpose / zero-pad / dtype cast only."""
    import ml_dtypes
    bf16 = np.dtype(ml_dtypes.bfloat16)
    x = np.asarray(x, dtype=np.float32)
    w_all = np.concatenate([np.asarray(Wq, np.float32),
                            np.asarray(Wk, np.float32),
                            np.asarray(Wv, np.float32)], axis=1)  # [C, 3C]
    w_t = np.ascontiguousarray(
        w_all.reshape(2, 128, 3 * C).transpose(1, 0, 2).reshape(128, 6 * C)
    ).astype(bf16)

    in_maps = []
    for d in range(NCORES):
        b, q = divmod(d, 4)
        quarter = x[b, 2048 * q:2048 * (q + 1), :]          # [2048, C]

        seg = 0 if q < 2 else 4096
        grp2 = x[b, seg:seg + 4096:2, :]                    # [2048, C]
        r0 = 1024 * (q % 2)
        bpre = grp2[0:1024] if r0 == 1024 else np.zeros((XB, C), np.float32)

        grp4 = x[b, 0:8192:4, :]                            # [2048, C]
        r0c = 512 * q
        cpre = np.concatenate(
            [grp4[0:r0c], np.zeros((XC - r0c, C), np.float32)], axis=0)

        slab = np.concatenate([quarter, bpre, cpre], axis=0)  # [XW, C]
        xt = np.ascontiguousarray(
            slab.T.reshape(2, 128, XW).transpose(1, 0, 2).reshape(128, 2 * XW)
        ).astype(bf16)

        bias = np.zeros((128, 5), np.float32)
        bias[:, 4] = -2.0
        bias[:, 0] = -2.0 if r0 == 1024 else NEG
        for g in range(3):
            bias[:, 1 + g] = -2.0 if g < q else NEG

        in_maps.append({"x": xt, "w": w_t, "bias": bias})
    return in_maps


def kernel(x, Wq, Wk, Wv):
    from concourse.bass_utils import run_bass_kernel_spmd

    nc = _get_program()
    in_maps = make_in_maps(x, Wq, Wk, Wv)
    res = run_bass_kernel_spmd(nc, in_maps, core_ids=list(range(NCORES)))
    out = np.empty((B, N, C), np.float32)
    for d in range(NCORES):
        b, q = divmod(d, 4)
        out[b, 2048 * q:2048 * (q + 1), :] = res.results[d]["out"]
    return out


# revision 13
# speedup vs baseline: 1.4063x; 1.0083x over previous
"""Dilated self-attention TRN2 Bass kernel, v2 (bf16, exact causal shapes).

Problem (hardcoded): B=2, N=8192, C=256, WS=[2048,4096,8192], RS=[1,2,4],
HEAD_IDX=0 -> G=7 groups of s=2048 rows each.

Sharding: core d = (b=d//4, q=d%4) owns output positions [2048q, 2048(q+1))
of batch b.  Jobs per core (all bf16 matmuls, 1 cycle/row at any width):
  C: 4 q-tiles (slots 12..15) of the r=4 group, prefix 12 kt (data-masked)
  B: 8 q-tiles (slots 8..15) of the r=2 half, prefix 8 kt (data-masked)
  A: 16 q-tiles full causal over own quarter
Layout tricks vs v1:
  - One bf16 x slab [128,2,4608] = [A 2048 | Bpre 1024 | Cpre 1536]; B/C
    diag K/Q are strided views into the A region (no re-projection).
  - V tiles for B/C diag are partition-gathered from A's V tiles via
    SBUF->SBUF DMA (stride 2/4) instead of re-projecting.
  - Exact per-q-tile causal prefixes (no 512-block rounding).
  - Scatter-add runs inside the output PSUM accumulation group: placement
    matmuls for B/C land before A's U matmuls; finalize reads PSUM once.
  - Emission order starts the C job while A/B projections still run.
"""

import numpy as np

B, N, C = 2, 8192, 256
S = 2048
NCORES = 8
SCALE = 0.0625
NEG = -1.0e9

XA, XB, XC = 2048, 1024, 1536          # slab section widths
XW = XA + XB + XC                      # 4608

_PROG = None


def _build_program():
    import concourse.mybir as mybir
    import concourse.tile as tile
    from concourse import bacc

    F32 = mybir.dt.float32
    BF16 = mybir.dt.bfloat16
    FP8 = mybir.dt.float8e4
    DR = mybir.MatmulPerfMode.DoubleRow
    Exp = mybir.ActivationFunctionType.Exp

    nc = bacc.Bacc("TRN2", target_bir_lowering=False, debug=False,
                   num_devices=NCORES)

    x_d = nc.dram_tensor("x", [128, 2 * XW], BF16, kind="ExternalInput")
    w_d = nc.dram_tensor("w", [128, 2 * 3 * C], BF16, kind="ExternalInput")
    bias_d = nc.dram_tensor("bias", [128, 5], F32, kind="ExternalInput")
    out_d = nc.dram_tensor("out", [S, C], F32, kind="ExternalOutput")

    with tile.TileContext(nc) as tc:
        with (
            tc.tile_pool(name="const", bufs=1) as cpool,
            tc.tile_pool(name="xsb", bufs=1) as xpool,
            tc.tile_pool(name="kqt", bufs=1) as kqpool,
            tc.tile_pool(name="vext", bufs=32) as vpool,
            tc.tile_pool(name="probs", bufs=8) as ppool,
            tc.tile_pool(name="probs2", bufs=8) as ppool2,
            tc.tile_pool(name="probs8", bufs=8) as ppool8,
            tc.tile_pool(name="stage", bufs=12) as spool,
            tc.tile_pool(name="fin", bufs=4) as fpool,
            tc.tile_pool(name="ps_s", bufs=3, space="PSUM") as ps_scores,
            tc.tile_pool(name="ps_u", bufs=2, space="PSUM") as ps_small,
        ):
            # ---- weights + early x chunks first (PE warmup) ----
            w_sb = cpool.tile([128, 2, 3 * C], BF16, tag="w")
            nc.sync.dma_start(w_sb[:], w_d.rearrange("p (c k) -> p c k", c=2))

            xb = xpool.tile([128, 2, XW], BF16, tag="xb")
            x_r = x_d.rearrange("p (c k) -> p c k", c=2)
            CH = 512
            CHUNK_ORDER = [0, 1, 6, 7, 8, 2, 3, 4, 5]
            for i, ch in enumerate(CHUNK_ORDER):
                eng = nc.sync if i % 2 == 0 else nc.gpsimd
                eng.dma_start(xb[:, :, CH * ch:CH * (ch + 1)],
                              x_r[:, :, CH * ch:CH * (ch + 1)])
            bias_t = cpool.tile([128, 5], F32, tag="bias")
            nc.sync.dma_start(bias_t[:], bias_d[:])

            def wap(name, ci, co):
                i = {"q": 0, "k": 1, "v": 2}[name]
                return w_sb[:, ci, 256 * i + 128 * co:256 * i + 128 * (co + 1)]

            # ---- constants ----
            ones_t = cpool.tile([128, 128], F32, tag="ones")
            nc.gpsimd.memset(ones_t[:], 1.0)
            # identity (stationary) and -1e9 strict-upper (moving): adding
            # ident.T @ trineg to a diag score tile applies the causal mask
            # inside the PSUM accumulation group, PE-side.
            id_f = cpool.tile([128, 128], F32, tag="id_f")
            nc.gpsimd.affine_select(
                out=id_f[:], in_=ones_t[:],
                compare_op=mybir.AluOpType.is_equal,
                fill=0.0, base=0, pattern=[[1, 128]], channel_multiplier=-1)
            ident = cpool.tile([128, 128], BF16, tag="ident")
            nc.vector.tensor_copy(ident[:], id_f[:])
            neg_t = cpool.tile([128, 128], F32, tag="negs")
            nc.gpsimd.memset(neg_t[:], NEG)
            tri_f = cpool.tile([128, 128], F32, tag="tri_f")
            # keep NEG where p > f (mask keys k > query q), 0 elsewhere
            nc.gpsimd.affine_select(
                out=tri_f[:], in_=neg_t[:],
                compare_op=mybir.AluOpType.is_ge,
                fill=0.0, base=-1, pattern=[[-1, 128]], channel_multiplier=1)
            trineg = cpool.tile([128, 128], BF16, tag="trineg")
            nc.vector.tensor_copy(trineg[:], tri_f[:])

            pmats = {}

            def build_pmats():
                for stride, u in [(2, 0), (2, 1), (4, 0), (4, 1),
                                  (4, 2), (4, 3)]:
                    pf = cpool.tile([128, 128], F32, tag="fscratch",
                                    name=f"pmf{stride}_{u}")
                    nc.gpsimd.affine_select(
                        out=pf[:], in_=ones_t[:],
                        compare_op=mybir.AluOpType.is_equal,
                        fill=0.0, base=128 * u,
                        pattern=[[1, 128]], channel_multiplier=-stride)
                    pm = cpool.tile([128, 128], BF16, tag=f"pm{stride}_{u}")
                    nc.vector.tensor_copy(pm[:], pf[:])
                    pmats[stride, u] = pm

            # ---- projection helpers (emitted lazily, in job-driven order) --
            ktb = kqpool.tile([128, 2, XA], BF16, tag="ktb")
            qtb = kqpool.tile([128, 2, XA], BF16, tag="qtb")
            kt8 = kqpool.tile([128, 2, XB + XC], FP8, tag="kt8")
            qt8 = kqpool.tile([128, 2, 1536], FP8, tag="qt8")
            kt8a = kqpool.tile([128, 2, XA], FP8, tag="kt8a")
            qt8a = kqpool.tile([128, 2, XA], FP8, tag="qt8a")
            _cp_flip = [0]

            def copy_to(dst, src):
                _cp_flip[0] ^= 1
                if _cp_flip[0]:
                    nc.scalar.copy(dst, src)
                else:
                    nc.vector.tensor_copy(dst, src)

            def kproj(ch):
                c0 = CH * ch
                for co in range(2):
                    ps = ps_small.tile([128, 512], F32, tag="proj", bufs=3)
                    for ci in range(2):
                        nc.tensor.matmul(
                            ps[:], wap("k", ci, co), xb[:, ci, c0:c0 + CH],
                            start=(ci == 0), stop=(ci == 1))
                    if c0 >= XA:
                        copy_to(kt8[:, co, c0 - XA:c0 - XA + CH], ps[:])
                    else:
                        copy_to(ktb[:, co, c0:c0 + CH], ps[:])

            def qproj(ch):
                c0 = CH * ch
                for co in range(2):
                    ps = ps_small.tile([128, 512], F32, tag="proj", bufs=3)
                    for ci in range(2):
                        nc.tensor.matmul(
                            ps[:], wap("q", ci, co), xb[:, ci, c0:c0 + CH],
                            start=(ci == 0), stop=(ci == 1))
                    copy_to(qtb[:, co, c0:c0 + CH], ps[:])
                    nc.gpsimd.tensor_copy(
                        qt8[:, co, c0 // 2:c0 // 2 + 256],
                        qtb[:, co, c0:c0 + CH:2])
                    nc.gpsimd.tensor_copy(
                        qt8[:, co, 1024 + c0 // 4:1024 + c0 // 4 + 128],
                        qtb[:, co, c0:c0 + CH:4])

            def vproj_pair(src_c0, name, dt=BF16):
                """project x cols [src_c0, src_c0+256) -> one [128,2,258]"""
                ps = ps_small.tile([128, 512], F32, tag="proj", bufs=3,
                                   name=f"psv{name}")
                for half in range(2):
                    for ci in range(2):
                        nc.tensor.matmul(
                            ps[:, 256 * half:256 * (half + 1)],
                            xb[:, ci,
                               src_c0 + 128 * half:src_c0 + 128 * (half + 1)],
                            w_sb[:, ci, 512:768],
                            start=(ci == 0), stop=(ci == 1))
                v = vpool.tile([128, 2, 258], dt, tag="vp", name=f"vp{name}")
                if dt is BF16:
                    copy_to(v[:, :, 0:256],
                            ps.rearrange("p (h c) -> p h c", h=2))
                else:
                    copy_to(v[:, :, 0:256],
                            ps.rearrange("p (h c) -> p h c", h=2))
                nc.vector.memset(v[:, :, 256:257], 1.0)
                nc.vector.memset(v[:, :, 257:258], 0.0)
                return v

            vA = [None] * 8
            vA8 = [None] * 8

            def vAp(j):
                if vA[j] is None:
                    vA[j] = vproj_pair(256 * j, f"A{j}")
                return vA[j]

            def vBd_gather(j):
                """B diag pair j rows = A rows [512j, 512j+512) step 2."""
                v = vpool.tile([128, 2, 258], BF16, tag="vp", name=f"vBd{j}")
                for half in range(2):
                    for s in range(2):
                        src = vAp(2 * j + half)
                        nc.sync.dma_start(
                            v[64 * s:64 * (s + 1), half, :],
                            src[0:128:2, s, :])
                return v

            def vCd_gather(j):
                """C diag pair j rows = A rows [1024j, 1024j+1024) step 4."""
                v = vpool.tile([128, 2, 258], BF16, tag="vp", name=f"vCd{j}")
                for half in range(2):
                    base = 512 * (2 * j + half)
                    for s in range(4):
                        src = vAp((base + 128 * s) // 256)
                        sh = ((base + 128 * s) // 128) % 2
                        nc.sync.dma_start(
                            v[32 * s:32 * (s + 1), half, :],
                            src[0:128:4, sh, :])
                return v

            jobs = {
                "C": dict(nq=4, P=12, stride=4, kpre=XA + XB, dstep=4),
                "B": dict(nq=8, P=8, stride=2, kpre=XA, dstep=2),
                "A": dict(nq=16, P=0, stride=1, kpre=None, dstep=1),
            }
            vslabs = {}

            def kstat(job, ci, kt):
                j = jobs[job]
                if kt < j["P"]:
                    c0 = j["kpre"] + 128 * kt
                    return ktb[:, ci, c0:c0 + 128]
                d = kt - j["P"]
                st = j["dstep"]
                return ktb[:, ci, 128 * st * d:128 * st * (d + 1):st]

            def qmov(job, ci, tau):
                st = jobs[job]["dstep"]
                return qtb[:, ci, 128 * st * tau:128 * st * (tau + 1):st]

            def kstat8(job, kt):
                if job == "A":
                    return kt8a[:, :, 128 * kt:128 * (kt + 1)]
                off = 0 if job == "B" else XB
                return kt8[:, :, off + 128 * kt:off + 128 * (kt + 1)]

            def qmov8(job, tau):
                if job == "A":
                    return qt8a[:, :, 128 * tau:128 * (tau + 1)]
                off = 0 if job == "B" else 1024
                return qt8[:, :, off + 128 * tau:off + 128 * (tau + 1)]

            def vmov(job, kt):
                j = vslabs[job][kt // 2]
                return j[:, kt % 2, :]

            st_tiles = {}

            def do_qtile(job, tau, ups):
                """scores+exp+U for q-tile tau of job; U accumulated into
                psum tile `ups` (A: group already started by placements)."""
                j = jobs[job]
                P = j["P"]
                nkt = P + tau + 1
                pbs = []            # (pb_tile, kt0, nkt_in_group, is8)
                for gi, g0 in enumerate(range(0, nkt, 4)):
                    gn = min(4, nkt - g0)
                    # groups fully inside the prefix use the fp8 DoubleRow
                    # path (long-range diffuse attention tolerates fp8)
                    if job == "A":
                        is8 = g0 + gn - 1 <= tau - 5
                    else:
                        is8 = g0 + gn <= P
                    ps = ps_scores.tile([128, 512], F32, tag="scores")
                    if is8:
                        for i in range(gn):
                            kt = g0 + i
                            nc.tensor.matmul(
                                ps[:, 128 * i:128 * (i + 1)],
                                kstat8(job, kt), qmov8(job, tau),
                                start=True, stop=True, perf_mode=DR)
                        if job == "A":
                            bias_ap = bias_t[:, 4:5]
                        else:
                            b0 = 0 if job == "B" else 1 + g0 // 4
                            bias_ap = bias_t[:, b0:b0 + 1]
                        pb = ppool8.tile([128, 512], FP8, tag="pb8")
                    else:
                        for i in range(gn):
                            kt = g0 + i
                            diag = kt == nkt - 1
                            reg = ps[:, 128 * i:128 * (i + 1)]
                            for ci in range(2):
                                nc.tensor.matmul(
                                    reg, kstat(job, ci, kt),
                                    qmov(job, ci, tau),
                                    start=(ci == 0),
                                    stop=(ci == 1 and not diag))
                            if diag:
                                nc.tensor.matmul(reg, ident[:], trineg[:],
                                                 start=False, stop=True)
                        if g0 < P:
                            b0 = 0 if job == "B" else 1 + g0 // 4
                            bias_ap = bias_t[:, b0:b0 + 1]
                        else:
                            bias_ap = bias_t[:, 4:5]
                        pb = (ppool if gi % 2 == 0 else ppool2).tile(
                            [128, 512], BF16, tag="pb")
                    pbs.append((pb, g0, gn, is8))
                    nc.scalar.activation(pb[:, 0:128 * gn], ps[:, 0:128 * gn],
                                         Exp, bias=bias_ap, scale=SCALE)
                for pb, g0, gn, is8 in pbs:
                    if is8:
                        for jj in range(gn // 2):
                            kt = g0 + 2 * jj
                            nc.tensor.matmul(
                                ups[:],
                                pb[:, 256 * jj:256 * (jj + 1)].rearrange(
                                    "p (two f) -> p two f", two=2),
                                (vA8 if job == "A"
                                 else vslabs[job])[kt // 2][:],
                                start=(kt == 0 and job != "A"),
                                stop=False, perf_mode=DR)
                    else:
                        for i in range(gn):
                            kt = g0 + i
                            nc.tensor.matmul(
                                ups[:], pb[:, 128 * i:128 * (i + 1)],
                                vmov(job, kt),
                                start=(kt == 0 and job != "A"),
                                stop=(kt == nkt - 1))

            def run_shared(job, tau):
                ups = ps_small.tile([128, 258], F32, tag="u",
                                    name=f"u{job}{tau}")[:]
                do_qtile(job, tau, ups)
                st = spool.tile([128, 258], BF16, tag="st",
                                name=f"st{job}{tau}")
                nc.vector.tensor_copy(st[:], ups[:])
                st_tiles[job, tau] = st

            # ================= emission schedule =================
            # projections needed for C first, C starts ASAP, then B, then A.
            kproj(0); qproj(0)
            vAp(0); vAp(1)
            kproj(1); qproj(1)
            vAp(2); vAp(3)
            for ch in (6, 7, 8):
                kproj(ch)
            vCpre = [vproj_pair(XA + XB + 256 * j, f"Cp{j}", FP8) for j in range(6)]
            vslabC = vCpre + [vCd_gather(0)]
            vslabs["C"] = vslabC           # vCd[1] appended below

            run_shared("C", 0)
            kproj(2); qproj(2)
            vAp(4); vAp(5)
            run_shared("C", 1)
            kproj(3); qproj(3)
            vAp(6); vAp(7)
            vslabC.append(vCd_gather(1))
            run_shared("C", 2)
            kproj(4); kproj(5)
            vBpre = [vproj_pair(XA + 256 * j, f"Bp{j}", FP8) for j in range(4)]
            run_shared("C", 3)
            vBd = [vBd_gather(j) for j in range(4)]
            vslabs["B"] = vBpre + vBd
            vslabs["A"] = vA
            build_pmats()

            # ---- B job interleaved with A (A tile t needs st_B[t//2]) ----
            out_r = out_d.rearrange("(t p) c -> t p c", p=128)

            def run_A(t):
                ups = ps_small.tile([128, 258], F32, tag="u",
                                    name=f"uA{t}")[:]
                nc.tensor.matmul(ups, pmats[4, t % 4][:],
                                 st_tiles["C", t // 4][:],
                                 start=True, stop=False)
                nc.tensor.matmul(ups, pmats[2, t % 2][:],
                                 st_tiles["B", t // 2][:],
                                 start=False, stop=False)
                do_qtile("A", t, ups)
                rec = fpool.tile([128, 1], F32, tag="rec")
                nc.vector.reciprocal(rec[:], ups[:, 256:257])
                ft = fpool.tile([128, 256], F32, tag="fin", bufs=6,
                                name=f"fin{t}")
                nc.vector.tensor_scalar_mul(ft[:], ups[:, 0:256], rec[:])
                nc.sync.dma_start(out_r[t], ft[:])

            # deferred fp8 twins of A-region K/Q/V (used by far A tiles,
            # first at A t=8 -> spread emission across early B iterations)
            def emit_twins(part):
                for ch in (part, part + 2):
                    for co in range(2):
                        c0 = CH * ch
                        nc.gpsimd.tensor_copy(kt8a[:, co, c0:c0 + CH],
                                              ktb[:, co, c0:c0 + CH])
                        nc.gpsimd.tensor_copy(qt8a[:, co, c0:c0 + CH],
                                              qtb[:, co, c0:c0 + CH])
                for j in (4 * part, 4 * part + 1, 4 * part + 2, 4 * part + 3):
                    v8 = vpool.tile([128, 2, 258], FP8, tag="vp",
                                    name=f"vA8_{j}")
                    nc.gpsimd.tensor_copy(v8[:], vA[j][:])
                    vA8[j] = v8

            for tau in range(8):
                run_shared("B", tau)
                if tau < 2:
                    emit_twins(tau)
                run_A(2 * tau)
                run_A(2 * tau + 1)

    nc.compile()
    return nc


def _get_program():
    global _PROG
    if _PROG is None:
        _PROG = _build_program()
    return _PROG


def make_in_maps(x, Wq, Wk, Wv):
    """Host-side sharding: gather / transpose / zero-pad / dtype cast only."""
    import ml_dtypes
    bf16 = np.dtype(ml_dtypes.bfloat16)
    x = np.asarray(x, dtype=np.float32)
    w_all = np.concatenate([np.asarray(Wq, np.float32),
                            np.asarray(Wk, np.float32),
                            np.asarray(Wv, np.float32)], axis=1)  # [C, 3C]
    w_t = np.ascontiguousarray(
        w_all.reshape(2, 128, 3 * C).transpose(1, 0, 2).reshape(128, 6 * C)
    ).astype(bf16)

    in_maps = []
    for d in range(NCORES):
        b, q = divmod(d, 4)
        quarter = x[b, 2048 * q:2048 * (q + 1), :]          # [2048, C]

        seg = 0 if q < 2 else 4096
        grp2 = x[b, seg:seg + 4096:2, :]                    # [2048, C]
        r0 = 1024 * (q % 2)
        bpre = grp2[0:1024] if r0 == 1024 else np.zeros((XB, C), np.float32)

        grp4 = x[b, 0:8192:4, :]                            # [2048, C]
        r0c = 512 * q
        cpre = np.concatenate(
            [grp4[0:r0c], np.zeros((XC - r0c, C), np.float32)], axis=0)

        slab = np.concatenate([quarter, bpre, cpre], axis=0)  # [XW, C]
        xt = np.ascontiguousarray(
            slab.T.reshape(2, 128, XW).transpose(1, 0, 2).reshape(128, 2 * XW)
        ).astype(bf16)

        bias = np.zeros((128, 5), np.float32)
        bias[:, 4] = -2.0
        bias[:, 0] = -2.0 if r0 == 1024 else NEG
        for g in range(3):
            bias[:, 1 + g] = -2.0 if g < q else NEG

        in_maps.append({"x": xt, "w": w_t, "bias": bias})
    return in_maps


def kernel(x, Wq, Wk, Wv):
    from concourse.bass_utils import run_bass_kernel_spmd

    nc = _get_program()
    in_maps = make_in_maps(x, Wq, Wk, Wv)
    res = run_bass_kernel_spmd(nc, in_maps, core_ids=list(range(NCORES)))
    out = np.empty((B, N, C), np.float32)
    for d in range(NCORES):
        b, q = divmod(d, 4)
        out[b, 2048 * q:2048 * (q + 1), :] = res.results[d]["out"]
    return out
